# revision 1
# baseline (speedup 1.0000x reference)
"""TRN2 Bass kernel for nn_GCNModel: GCNConv + global max pool + root MLP head.

Strategy (8 NeuronCores, SPMD):
  - Graphs are assigned to cores (16 each, snake order by size for balance).
    Each core owns its graphs' nodes (contiguous ranges of the sorted batch).
  - h' = (x @ W1) * rsqrt(deg) computed shard-local in bf16 (PE, fp32 PSUM),
    then AllGather'd into a replicated row table [NT, 128] bf16 in DRAM.
  - Edge aggregation per core: edges (+ self loops) bucketed by
    (dst 128-node window, src table half), gathered row-wise from the table
    via gpsimd dma_gather (int16 half-local indices), then scattered into
    per-window PSUM accumulators with one-hot matmuls (exact, fp32 accum).
  - hx = relu(dinv * agg + b1) masked with -1e30 on pad rows; per-window
    column max after a PE transpose; per-graph max over its windows.
  - Head: news = relu(x_root @ W0 + b0); z = [pooled|news] @ Wl1 -> relu ->
    @ Wl2 -> log_softmax. All fp32.

The chunk schedule is made SPMD-uniform by padding per-(window,half) chunk
counts to the max over cores; pad lanes use dst column 200 (one-hot miss) so
they contribute exactly zero.
"""
import sys
import types
import contextlib
import ctypes

import numpy as np
import ml_dtypes

NCORES = 8
P = 128
OPCHUNKS = 8           # chunks (of 128 slots) per dma_gather op (HW limit ~1024 idxs/op)
TRACE = False          # set by test.py for profiling
LAST_EXEC_NS = None


# ---------------------------------------------------------------------------
# axon NTFF profile hook (the image's antenv lacks axon_hooks)
# ---------------------------------------------------------------------------
def _install_ntff_hook():
    if "antenv.axon_hooks" in sys.modules:
        return
    try:
        lib = ctypes.CDLL("/opt/axon/libaxon_pjrt.so")
        if not hasattr(lib, "axon_start_nrt_profile"):
            return
    except OSError:
        return
    lib.axon_start_nrt_profile.argtypes = [ctypes.POINTER(ctypes.c_int64), ctypes.c_size_t]
    lib.axon_start_nrt_profile.restype = ctypes.c_int64
    lib.axon_stop_nrt_profile.argtypes = [ctypes.c_char_p]
    lib.axon_stop_nrt_profile.restype = ctypes.c_int64

    @contextlib.contextmanager
    def _hook(output_dir, device_ids):
        import jax
        jax.devices()
        if device_ids:
            ids = (ctypes.c_int64 * len(device_ids))(*device_ids)
            rc = lib.axon_start_nrt_profile(ids, len(device_ids))
        else:
            rc = lib.axon_start_nrt_profile(None, 0)
        if rc != 0:
            raise RuntimeError(f"axon_start_nrt_profile rc={rc}")
        try:
            yield
        finally:
            n = lib.axon_stop_nrt_profile(str(output_dir).encode())
            print(f"ntff profile: {n} file(s) -> {output_dir}", file=sys.stderr)

    mod = types.ModuleType("antenv.axon_hooks")
    mod.get_axon_ntff_profile_hook = lambda: _hook
    mod.set_axon_ntff_profile_hook = lambda h: None
    sys.modules["antenv.axon_hooks"] = mod


# ---------------------------------------------------------------------------
# host-side schedule
# ---------------------------------------------------------------------------
def build_schedule(x, edge_index, batch, num_graphs):
    N = x.shape[0]
    G = int(num_graphs)
    src = np.asarray(edge_index[0], dtype=np.int64)
    dst = np.asarray(edge_index[1], dtype=np.int64)
    batch = np.asarray(batch, dtype=np.int64)

    sizes = np.bincount(batch, minlength=G)
    gstart = np.zeros(G + 1, np.int64)
    np.cumsum(sizes, out=gstart[1:])

    first_idx = np.full(G, np.iinfo(np.int32).max, np.int64)
    np.minimum.at(first_idx, batch, np.arange(N))
    first_idx = np.clip(first_idx, 0, N - 1)

    deg = (np.bincount(dst, minlength=N) + 1).astype(np.float32)

    assert G % NCORES == 0, "graph count must divide core count"
    GPC = G // NCORES
    order = np.argsort(-sizes, kind="stable")
    g2core = np.zeros(G, np.int64)
    g2slot = np.zeros(G, np.int64)
    for i, g in enumerate(order):
        rnd, pos = divmod(i, NCORES)
        core = pos if rnd % 2 == 0 else NCORES - 1 - pos
        g2core[g] = core
        g2slot[g] = rnd

    S_GRAPH = max(512, int(np.ceil(sizes.max() / P)) * P)
    WPG = S_GRAPH // P
    NWIN = GPC * WPG

    core_graphs = []
    for c in range(NCORES):
        gs = [g for g in range(G) if g2core[g] == c]
        gs.sort(key=lambda g: g2slot[g])
        core_graphs.append(gs)
    real_c = np.array([sizes[core_graphs[c]].sum() for c in range(NCORES)])
    S_SHARD = int(np.ceil(real_c.max() / P)) * P
    NT = S_SHARD * NCORES
    HALF = NT // 2
    assert HALF % P == 0 and HALF < 32768, f"half table {HALF} must fit int16"

    shard_row = np.zeros(N, np.int64)
    pad_w = np.zeros(N, np.int64)
    pad_col = np.zeros(N, np.int64)
    node_core = g2core[batch]
    for c in range(NCORES):
        pos = 0
        for g in core_graphs[c]:
            n0, n1 = gstart[g], gstart[g + 1]
            cnt = n1 - n0
            ar = np.arange(cnt)
            shard_row[n0:n1] = c * S_SHARD + pos + ar
            slot = g2slot[g]
            pad_w[n0:n1] = slot * WPG + ar // P
            pad_col[n0:n1] = ar % P
            pos += cnt

    # edge + self-loop lists per core
    ecore = node_core[dst]
    t_rows, t_wins, t_cols = [], [], []
    for c in range(NCORES):
        m = ecore == c
        nm = node_core == c
        t_rows.append(np.concatenate([shard_row[src[m]], shard_row[nm]]))
        t_wins.append(np.concatenate([pad_w[dst[m]], pad_w[nm]]))
        t_cols.append(np.concatenate([pad_col[dst[m]], pad_col[nm]]))

    counts = np.zeros((NCORES, NWIN, 2), np.int64)
    for c in range(NCORES):
        h = (t_rows[c] >= HALF).astype(np.int64)
        np.add.at(counts, (c, t_wins[c], h), 1)
    cmax = counts.max(axis=0)
    chunks_wh = np.ceil(cmax / P).astype(np.int64)
    chunks_wh[:, 0] = np.maximum(chunks_wh[:, 0], 1)  # >=1 chunk so PSUM is written
    slots_wh = chunks_wh * P
    half_len = [int(slots_wh[:, h].sum()) for h in (0, 1)]
    TOTC = int(chunks_wh.sum())
    # per-half slot offset of each window's segment
    seg_off = np.zeros((NWIN, 2), np.int64)
    seg_off[1:, 0] = np.cumsum(slots_wh[:-1, 0])
    seg_off[1:, 1] = np.cumsum(slots_wh[:-1, 1])

    src16 = [[None, None] for _ in range(NCORES)]
    dstcol = np.full((NCORES, P, TOTC), 200.0, np.float32)
    for c in range(NCORES):
        rows, wins, cols = t_rows[c], t_wins[c], t_cols[c]
        h = (rows >= HALF).astype(np.int64)
        ci = 0
        for hh in (0, 1):
            stream = np.zeros(half_len[hh], np.int64)
            base = 0
            for w in range(NWIN):
                m = (wins == w) & (h == hh)
                k = int(m.sum())
                stream[base:base + k] = rows[m] - hh * HALF
                base += int(slots_wh[w, hh])
            src16[c][hh] = stream.astype(np.int16)
        # dstcol columns in window-major chunk order: for w: for hh: for k
        for w in range(NWIN):
            for hh in (0, 1):
                m = (wins == w) & (h == hh)
                cw = cols[m]
                for k in range(int(chunks_wh[w, hh])):
                    seg = cw[k * P:(k + 1) * P]
                    dstcol[c, :len(seg), ci] = seg
                    ci += 1
        assert ci == TOTC

    def wrap16(v):
        m = v.reshape(-1, 16).T  # [16, S]
        return np.ascontiguousarray(np.tile(m, (8, 1)).astype(np.int16))

    idx_w = [np.stack([wrap16(src16[c][hh]) for c in range(NCORES)]) for hh in (0, 1)]

    F = x.shape[1]
    x_core = np.zeros((NCORES, S_SHARD, F), np.float32)
    deg_padT = np.ones((NCORES, NWIN, P), np.float32)
    deg_shard = np.ones((NCORES, P, S_SHARD // P), np.float32)
    deg_pad = np.ones((NCORES, P, NWIN), np.float32)
    maskneg = np.full((NCORES, P, NWIN), -1e30, np.float32)
    rootsT = np.zeros((NCORES, F, GPC), np.float32)
    xf = np.asarray(x, dtype=np.float32)
    for c in range(NCORES):
        pos = 0
        for g in core_graphs[c]:
            n0, n1 = gstart[g], gstart[g + 1]
            cnt = n1 - n0
            x_core[c, pos:pos + cnt] = xf[n0:n1]
            pos += cnt
        nm = node_core == c
        sr = shard_row[nm] - c * S_SHARD
        d = deg[nm]
        deg_shard[c, sr % P, sr // P] = d
        deg_pad[c, pad_col[nm], pad_w[nm]] = d
        deg_padT[c, pad_w[nm], pad_col[nm]] = d
        maskneg[c, pad_col[nm], pad_w[nm]] = 0.0
        for j, g in enumerate(core_graphs[c]):
            rootsT[c, :, j] = xf[first_idx[g]]

    out_map = np.array([core_graphs[c] for c in range(NCORES)])

    return dict(
        G=G, GPC=GPC, F=F, S_GRAPH=S_GRAPH, WPG=WPG, NWIN=NWIN,
        S_SHARD=S_SHARD, NT=NT, HALF=HALF,
        chunks_wh=chunks_wh, slots_wh=slots_wh, seg_off=seg_off,
        half_len=half_len, TOTC=TOTC,
        idxA=idx_w[0], idxB=idx_w[1], dstcol=dstcol,
        x_core=x_core, deg_shard=deg_shard, deg_pad=deg_pad,
        deg_padT=deg_padT,
        maskneg=maskneg, rootsT=rootsT, out_map=out_map,
    )


# ---------------------------------------------------------------------------
# bass program
# ---------------------------------------------------------------------------
def build_program(s, NCLS, need_mask):
    import concourse.bass as bass
    import concourse.bacc as bacc
    import concourse.tile as tile
    import concourse.mybir as mybir

    f32 = mybir.dt.float32
    bf16 = mybir.dt.bfloat16
    i16 = mybir.dt.int16
    AF = mybir.ActivationFunctionType
    ALU = mybir.AluOpType
    AX = mybir.AxisListType

    F, GPC, NWIN, WPG = s["F"], s["GPC"], s["NWIN"], s["WPG"]
    S_SHARD, NT, HALF, TOTC = s["S_SHARD"], s["NT"], s["HALF"], s["TOTC"]
    NKT = F // P                 # k chunks for the 768-dim contraction
    NST = S_SHARD // P           # shard tiles
    chunks_wh = s["chunks_wh"]
    slots_wh = s["slots_wh"]
    seg_off = s["seg_off"]
    half_len = s["half_len"]

    nc = bacc.Bacc("TRN2", target_bir_lowering=False, debug=False,
                   enable_asserts=False, num_devices=NCORES)

    OHG = 8  # chunks per batched one-hot build

    # inputs
    t_xT = nc.dram_tensor("xT_core", [F, S_SHARD], f32, kind="ExternalInput")
    t_W1 = nc.dram_tensor("W1", [F, P], f32, kind="ExternalInput")
    t_W0 = nc.dram_tensor("W0", [F, P], f32, kind="ExternalInput")
    t_Wl1 = nc.dram_tensor("Wl1", [2 * P, P], f32, kind="ExternalInput")
    t_Wl2 = nc.dram_tensor("Wl2", [P, NCLS], f32, kind="ExternalInput")
    t_b1 = nc.dram_tensor("b1_rep", [P, P], f32, kind="ExternalInput")
    t_b0 = nc.dram_tensor("b0_rep", [P, P], f32, kind="ExternalInput")
    t_bl1 = nc.dram_tensor("bl1_rep", [P, P], f32, kind="ExternalInput")
    t_bl2 = nc.dram_tensor("bl2_rep", [P, NCLS], f32, kind="ExternalInput")
    t_degs = nc.dram_tensor("deg_shard", [P, NST], f32, kind="ExternalInput")
    t_degp = nc.dram_tensor("deg_pad", [P, NWIN], f32, kind="ExternalInput")
    t_degpT = nc.dram_tensor("deg_padT", [1, NWIN * P], f32, kind="ExternalInput")
    t_mask = nc.dram_tensor("maskneg", [P, NWIN], f32, kind="ExternalInput")
    t_roots = nc.dram_tensor("rootsT", [F, GPC], f32, kind="ExternalInput")
    t_idxA = nc.dram_tensor("idxA", [P, half_len[0] // 16], i16, kind="ExternalInput")
    t_idxB = nc.dram_tensor("idxB", [P, half_len[1] // 16], i16, kind="ExternalInput")
    t_dcol = nc.dram_tensor("dstcol", [P, TOTC], f32, kind="ExternalInput")
    t_iota = nc.dram_tensor("iota_row", [P, P], f32, kind="ExternalInput")
    t_iota8 = nc.dram_tensor("iota8", [P, OHG * P], f32, kind="ExternalInput")
    t_ident = nc.dram_tensor("identity", [P, P], f32, kind="ExternalInput")
    t_y = nc.dram_tensor("y", [GPC, NCLS], f32, kind="ExternalOutput")

    with tile.TileContext(nc) as tc:
        with tc.tile_pool(name="const", bufs=1) as cst, \
             tc.tile_pool(name="work", bufs=2) as wk, \
             tc.tile_pool(name="small", bufs=3) as sm, \
             tc.tile_pool(name="gather", bufs=1) as gp, \
             tc.tile_pool(name="psA", bufs=2, space="PSUM") as psA, \
             tc.tile_pool(name="psB", bufs=2, space="PSUM") as psB, \
             tc.tile_pool(name="dram", bufs=1, space="DRAM") as dram:

            # ---- constants in SBUF ----
            ident_f = cst.tile([P, P], f32)
            nc.sync.dma_start(ident_f[:], t_ident[:])
            ident_bf = cst.tile([P, P], bf16)
            nc.vector.tensor_copy(ident_bf[:], ident_f[:])
            iota_sb = cst.tile([P, P], f32)
            nc.sync.dma_start(iota_sb[:], t_iota[:])
            iota8_sb = cst.tile([P, OHG * P], f32)
            nc.sync.dma_start(iota8_sb[:], t_iota8[:])
            b1_sb = cst.tile([P, P], f32)
            nc.sync.dma_start(b1_sb[:], t_b1[:])
            b0_sb = cst.tile([P, P], f32)
            nc.sync.dma_start(b0_sb[:], t_b0[:])
            bl1_sb = cst.tile([P, P], f32)
            nc.sync.dma_start(bl1_sb[:], t_bl1[:])
            bl2_sb = cst.tile([P, NCLS], f32)
            nc.sync.dma_start(bl2_sb[:], t_bl2[:])
            dcol_sb = cst.tile([P, TOTC], f32)
            nc.sync.dma_start(dcol_sb[:], t_dcol[:])
            idxA_sb = cst.tile([P, half_len[0] // 16], i16)
            nc.sync.dma_start(idxA_sb[:], t_idxA[:])
            idxB_sb = cst.tile([P, half_len[1] // 16], i16)
            nc.sync.dma_start(idxB_sb[:], t_idxB[:])

            # dinv arrays: 1/sqrt(deg) = sqrt(1/deg)
            degs_sb = cst.tile([P, NST], f32)
            nc.sync.dma_start(degs_sb[:], t_degs[:])
            dinvs_sb = cst.tile([P, NST], f32)
            nc.vector.reciprocal(dinvs_sb[:], degs_sb[:])
            nc.scalar.activation(dinvs_sb[:], dinvs_sb[:], AF.Sqrt)
            degp_sb = cst.tile([P, NWIN], f32)
            nc.sync.dma_start(degp_sb[:], t_degp[:])
            dinvp_sb = cst.tile([P, NWIN], f32)
            nc.vector.reciprocal(dinvp_sb[:], degp_sb[:])
            nc.scalar.activation(dinvp_sb[:], dinvp_sb[:], AF.Sqrt)
            mask_sb = cst.tile([P, NWIN], f32)
            nc.sync.dma_start(mask_sb[:], t_mask[:])
            # sqrt(deg) per (window, col) transposed + b1 row, both bf16, for
            # the K=1 bias matmul injecting b1*sqrt(deg) into each window PSUM
            degpT_f = cst.tile([1, NWIN * P], f32)
            nc.sync.dma_start(degpT_f[:], t_degpT[:])
            nc.scalar.activation(degpT_f[:], degpT_f[:], AF.Sqrt)
            sdegT_bf = cst.tile([1, NWIN * P], bf16)
            nc.vector.tensor_copy(sdegT_bf[:], degpT_f[:])
            b1row_bf = cst.tile([1, P], bf16)
            nc.vector.tensor_copy(b1row_bf[:], b1_sb[:1, :])

            # W1 as bf16 k-chunk tiles
            W1_bf = []
            for kc in range(NKT):
                wt = cst.tile([P, P], f32, tag="w1f")
                nc.sync.dma_start(wt[:], t_W1[kc * P:(kc + 1) * P, :])
                wb = cst.tile([P, P], bf16, tag=f"w1b{kc}")
                nc.vector.tensor_copy(wb[:], wt[:])
                W1_bf.append(wb)

            # ---- phase 1: h' shard (x pre-transposed on host) ----
            h_in = dram.tile([S_SHARD, P], bf16)
            h_full = dram.tile([NT, P], bf16, addr_space="Shared")
            with tc.tile_pool(name="xstr", bufs=1) as xsp:
                xbs = []
                for kc in range(NKT):
                    xf = xsp.tile([P, S_SHARD], f32, tag="xf")
                    nc.sync.dma_start(xf[:], t_xT[kc * P:(kc + 1) * P, :])
                    xb = xsp.tile([P, S_SHARD], bf16, tag=f"xb{kc}")
                    nc.vector.tensor_copy(xb[:], xf[:])
                    xbs.append(xb)
                for t in range(NST):
                    hps = psB.tile([P, P], f32, tag="acc")
                    for kc in range(NKT):
                        nc.tensor.matmul(hps[:], lhsT=xbs[kc][:, t * P:(t + 1) * P],
                                         rhs=W1_bf[kc][:],
                                         start=(kc == 0), stop=(kc == NKT - 1))
                    hp = sm.tile([P, P], bf16, tag="hp")
                    nc.vector.tensor_scalar(out=hp[:], in0=hps[:],
                                            scalar1=dinvs_sb[:, t:t + 1], scalar2=None,
                                            op0=ALU.mult)
                    nc.sync.dma_start(h_in[t * P:(t + 1) * P, :], hp[:])

            # ---- allgather ----
            nc.gpsimd.collective_compute(
                "AllGather", ALU.bypass,
                replica_groups=[list(range(NCORES))],
                ins=[h_in.opt()],
                outs=[h_full.opt()],
            )

            # ---- phase 2: edge aggregation ----
            # gather ops per half: list of (chunk_base_slot, nchunks)
            def half_ops(L):
                ops = []
                base = 0
                while base < L:
                    n = min(OPCHUNKS * P, L - base)
                    ops.append((base, n))
                    base += n
                return ops

            opsA = half_ops(half_len[0])
            opsB = half_ops(half_len[1])
            gtiles = {0: {}, 1: {}}
            idx_sb = {0: idxA_sb, 1: idxB_sb}
            tabs = {0: h_full[0:HALF, :], 1: h_full[HALF:NT, :]}

            def issue_gather(hh, opi, base, nsl):
                g = gp.tile([P, OPCHUNKS * P], bf16, tag=f"g{hh}")
                nc.gpsimd.dma_gather(
                    g[:, :nsl].rearrange("p (c f) -> p c f", f=P),
                    tabs[hh],
                    idx_sb[hh][:, base // 16: (base + nsl) // 16],
                    nsl, nsl, P,
                )
                gtiles[hh][opi] = g

            for opi, (base, nsl) in enumerate(opsA):
                issue_gather(0, opi, base, nsl)
            for opi, (base, nsl) in enumerate(opsB):
                issue_gather(1, opi, base, nsl)

            # batched one-hot builds: one DVE op per OHG chunks
            oh_tiles = {}

            def onehot_group(g0):
                n = min(OHG, TOTC - g0)
                oh = sm.tile([P, OHG * P], bf16, tag="oh")
                nc.vector.tensor_tensor(
                    out=oh[:, :n * P].rearrange("p (c f) -> p c f", f=P),
                    in0=iota8_sb[:, :n * P].rearrange("p (c f) -> p c f", f=P),
                    in1=dcol_sb[:, g0:g0 + n].to_broadcast([P, n, P]),
                    op=ALU.is_equal)
                oh_tiles[g0] = oh

            winmax_sb = cst.tile([P, NWIN], f32)
            TRW = 4  # windows per transpose/reduce batch
            ci = 0
            for w in range(NWIN):
                if w % TRW == 0:
                    tr = psA.tile([P, TRW * P], bf16, tag="tp")
                agg = psB.tile([P, P], f32, tag="acc")
                # bias chunk: agg += sqrt(deg)[col] * b1[f]
                nc.tensor.matmul(agg[:], lhsT=sdegT_bf[:1, w * P:(w + 1) * P],
                                 rhs=b1row_bf[:1, :], start=True, stop=False)
                nch = int(chunks_wh[w, 0] + chunks_wh[w, 1])
                j = 0
                for hh in (0, 1):
                    for k in range(int(chunks_wh[w, hh])):
                        slot = int(seg_off[w, hh]) + k * P
                        opi, off = divmod(slot, OPCHUNKS * P)
                        g = gtiles[hh][opi]
                        if ci % OHG == 0:
                            onehot_group(ci)
                        oh = oh_tiles[(ci // OHG) * OHG]
                        nc.tensor.matmul(agg[:], lhsT=oh[:, (ci % OHG) * P:(ci % OHG + 1) * P],
                                         rhs=g[:, off:off + P],
                                         start=False, stop=(j == nch - 1))
                        ci += 1
                        j += 1
                # hx = relu(dinv * (agg + sqrt(deg)*b1)) = relu(dinv*agg + b1)
                hx = sm.tile([P, P], bf16, tag="hx")
                nc.scalar.activation(hx[:], agg[:], AF.Relu,
                                     scale=dinvp_sb[:, w:w + 1])
                if need_mask:
                    nc.vector.tensor_scalar(out=hx[:], in0=hx[:],
                                            scalar1=mask_sb[:, w:w + 1], scalar2=None,
                                            op0=ALU.add)
                nc.tensor.transpose(tr[:, (w % TRW) * P:(w % TRW + 1) * P],
                                    hx[:], ident_bf[:])
                if w % TRW == TRW - 1:
                    nc.vector.reduce_max(
                        out=winmax_sb[:, w - TRW + 1:w + 1],
                        in_=tr[:].rearrange("p (c f) -> p c f", f=P), axis=AX.X)
            assert ci == TOTC

            # ---- pooling: per-graph max over its windows ----
            pooled_sb = cst.tile([P, GPC], f32)
            for g in range(GPC):
                nc.vector.reduce_max(out=pooled_sb[:, g:g + 1],
                                     in_=winmax_sb[:, g * WPG:(g + 1) * WPG], axis=AX.X)

            # ---- news = relu(x_root @ W0 + b0) ----
            nps = psB.tile([GPC, P], f32, tag="acc")
            for kc in range(NKT):
                rt = sm.tile([P, GPC], f32, tag="rt")
                nc.sync.dma_start(rt[:], t_roots[kc * P:(kc + 1) * P, :])
                w0t = sm.tile([P, P], f32, tag="w0t")
                nc.sync.dma_start(w0t[:], t_W0[kc * P:(kc + 1) * P, :])
                nc.tensor.matmul(nps[:], lhsT=rt[:], rhs=w0t[:],
                                 start=(kc == 0), stop=(kc == NKT - 1))
            news = sm.tile([GPC, P], f32, tag="news")
            nc.vector.tensor_add(news[:], nps[:], b0_sb[:GPC, :])
            nc.scalar.activation(news[:], news[:], AF.Relu)
            ntr = psA.tile([P, GPC], f32, tag="tp")
            nc.tensor.transpose(ntr[:], news[:], ident_f[:GPC, :GPC])
            newsT = sm.tile([P, GPC], f32, tag="newsT")
            nc.vector.tensor_copy(newsT[:], ntr[:])

            # ---- z = relu([pooled|news] @ Wl1 + bl1) ----
            wl1a = sm.tile([P, P], f32, tag="wl1a")
            nc.sync.dma_start(wl1a[:], t_Wl1[0:P, :])
            wl1b = sm.tile([P, P], f32, tag="wl1b")
            nc.sync.dma_start(wl1b[:], t_Wl1[P:2 * P, :])
            zps = psB.tile([GPC, P], f32, tag="acc")
            nc.tensor.matmul(zps[:], lhsT=pooled_sb[:], rhs=wl1a[:], start=True, stop=False)
            nc.tensor.matmul(zps[:], lhsT=newsT[:], rhs=wl1b[:], start=False, stop=True)
            z2 = sm.tile([GPC, P], f32, tag="z2")
            nc.vector.tensor_add(z2[:], zps[:], bl1_sb[:GPC, :])
            nc.scalar.activation(z2[:], z2[:], AF.Relu)
            ztr = psA.tile([P, GPC], f32, tag="tp")
            nc.tensor.transpose(ztr[:], z2[:], ident_f[:GPC, :GPC])
            z2T = sm.tile([P, GPC], f32, tag="z2T")
            nc.vector.tensor_copy(z2T[:], ztr[:])

            # ---- logits + log_softmax ----
            wl2 = sm.tile([P, NCLS], f32, tag="wl2")
            nc.sync.dma_start(wl2[:], t_Wl2[:])
            lps = psB.tile([GPC, NCLS], f32, tag="acc")
            nc.tensor.matmul(lps[:], lhsT=z2T[:], rhs=wl2[:], start=True, stop=True)
            lg = sm.tile([GPC, NCLS], f32, tag="lg")
            nc.vector.tensor_add(lg[:], lps[:], bl2_sb[:GPC, :])
            mx = sm.tile([GPC, 1], f32, tag="mx")
            nc.vector.reduce_max(out=mx[:], in_=lg[:], axis=AX.X)
            tt = sm.tile([GPC, NCLS], f32, tag="tt")
            nc.vector.tensor_scalar(out=tt[:], in0=lg[:], scalar1=mx[:],
                                    scalar2=None, op0=ALU.subtract)
            ee = sm.tile([GPC, NCLS], f32, tag="ee")
            nc.scalar.activation(ee[:], tt[:], AF.Exp)
            ss = sm.tile([GPC, 1], f32, tag="ss")
            nc.vector.reduce_sum(out=ss[:], in_=ee[:], axis=AX.X)
            ls = sm.tile([GPC, 1], f32, tag="ls")
            nc.scalar.activation(ls[:], ss[:], AF.Ln)
            yy = sm.tile([GPC, NCLS], f32, tag="yy")
            nc.vector.tensor_scalar(out=yy[:], in0=tt[:], scalar1=ls[:],
                                    scalar2=None, op0=ALU.subtract)
            nc.sync.dma_start(t_y[:], yy[:])

    nc.compile()
    return nc


# ---------------------------------------------------------------------------
# entry point
# ---------------------------------------------------------------------------
def kernel(**inputs) -> np.ndarray:
    global LAST_EXEC_NS
    _install_ntff_hook()
    from concourse import bass_utils
    from concourse.bass_interp import get_hw_module

    x = np.asarray(inputs["x"], dtype=np.float32)
    ei = np.asarray(inputs["edge_index"])
    batch = np.asarray(inputs["batch"])
    G = int(np.asarray(inputs["num_graphs"]))
    W1 = np.asarray(inputs["W1"], dtype=np.float32)
    b1 = np.asarray(inputs["b1"], dtype=np.float32)
    W0 = np.asarray(inputs["W0"], dtype=np.float32)
    b0 = np.asarray(inputs["b0"], dtype=np.float32)
    Wl1 = np.asarray(inputs["Wl1"], dtype=np.float32)
    bl1 = np.asarray(inputs["bl1"], dtype=np.float32)
    Wl2 = np.asarray(inputs["Wl2"], dtype=np.float32)
    bl2 = np.asarray(inputs["bl2"], dtype=np.float32)
    NCLS = Wl2.shape[1]

    s = build_schedule(x, ei, batch, G)
    need_mask = bool((b1 > 0).any())
    nc = build_program(s, NCLS, need_mask)

    rep = lambda v, n: np.ascontiguousarray(np.tile(v[None, :], (n, 1)).astype(np.float32))
    iota_row = rep(np.arange(P, dtype=np.float32), P)
    iota8 = np.ascontiguousarray(np.tile(iota_row, (1, 8)))
    ident = np.eye(P, dtype=np.float32)

    in_maps = []
    for c in range(NCORES):
        in_maps.append({
            "xT_core": np.ascontiguousarray(s["x_core"][c].T),
            "W1": W1, "W0": W0, "Wl1": Wl1, "Wl2": Wl2,
            "b1_rep": rep(b1, P), "b0_rep": rep(b0, P),
            "bl1_rep": rep(bl1, P), "bl2_rep": rep(bl2, P),
            "deg_shard": s["deg_shard"][c], "deg_pad": s["deg_pad"][c],
            "deg_padT": np.ascontiguousarray(s["deg_padT"][c].reshape(1, -1)),
            "maskneg": s["maskneg"][c], "rootsT": s["rootsT"][c],
            "idxA": s["idxA"][c], "idxB": s["idxB"][c],
            "dstcol": s["dstcol"][c],
            "iota_row": iota_row, "iota8": iota8, "identity": ident,
        })

    nc.m = get_hw_module(nc.m)
    res = bass_utils.run_bass_kernel_spmd(
        nc, in_maps, core_ids=list(range(NCORES)), trace=TRACE)
    LAST_EXEC_NS = res.exec_time_ns

    out = np.zeros((G, NCLS), np.float32)
    for c in range(NCORES):
        out[s["out_map"][c]] = res.results[c]["y"]
    return out



# revision 12
# speedup vs baseline: 2.5698x; 2.5698x over previous
"""TRN2 Bass kernel for nn_GCNModel: GCNConv + global max pool + root MLP head.

Strategy (8 NeuronCores, SPMD):
  - Graphs are assigned to cores (16 each, snake order by size for balance).
    Each core owns its graphs' nodes (contiguous ranges of the sorted batch).
  - h' = (x @ W1) * rsqrt(deg) computed shard-local in bf16 (PE, fp32 PSUM),
    then AllGather'd into a replicated row table [NT, 128] bf16 in DRAM.
  - Edge aggregation per core: edges (+ self loops) bucketed by
    (dst 128-node window, src table half), gathered row-wise from the table
    via gpsimd dma_gather (int16 half-local indices), then scattered into
    per-window PSUM accumulators with one-hot matmuls (exact, fp32 accum).
  - hx = relu(dinv * agg + b1) masked with -1e30 on pad rows; per-window
    column max after a PE transpose; per-graph max over its windows.
  - Head: news = relu(x_root @ W0 + b0); z = [pooled|news] @ Wl1 -> relu ->
    @ Wl2 -> log_softmax. All fp32.

The chunk schedule is made SPMD-uniform by padding per-(window,half) chunk
counts to the max over cores; pad lanes use dst column 200 (one-hot miss) so
they contribute exactly zero.
"""
import sys
import types
import contextlib
import ctypes

import numpy as np
import ml_dtypes

NCORES = 8
P = 128
OPCHUNKS = 8           # chunks (of 128 slots) per dma_gather op (1024 descs fits the ring)
NQUEUES = 4            # SWDGE queues; gathers round-robin so transfers overlap
GBUFS = 6              # in-flight gather buffers per table half
TRACE = False          # set by test.py for profiling
LAST_EXEC_NS = None


# ---------------------------------------------------------------------------
# axon NTFF profile hook (the image's antenv lacks axon_hooks)
# ---------------------------------------------------------------------------
def _install_ntff_hook():
    if "antenv.axon_hooks" in sys.modules:
        return
    try:
        lib = ctypes.CDLL("/opt/axon/libaxon_pjrt.so")
        if not hasattr(lib, "axon_start_nrt_profile"):
            return
    except OSError:
        return
    lib.axon_start_nrt_profile.argtypes = [ctypes.POINTER(ctypes.c_int64), ctypes.c_size_t]
    lib.axon_start_nrt_profile.restype = ctypes.c_int64
    lib.axon_stop_nrt_profile.argtypes = [ctypes.c_char_p]
    lib.axon_stop_nrt_profile.restype = ctypes.c_int64

    @contextlib.contextmanager
    def _hook(output_dir, device_ids):
        import jax
        jax.devices()
        if device_ids:
            ids = (ctypes.c_int64 * len(device_ids))(*device_ids)
            rc = lib.axon_start_nrt_profile(ids, len(device_ids))
        else:
            rc = lib.axon_start_nrt_profile(None, 0)
        if rc != 0:
            raise RuntimeError(f"axon_start_nrt_profile rc={rc}")
        try:
            yield
        finally:
            n = lib.axon_stop_nrt_profile(str(output_dir).encode())
            print(f"ntff profile: {n} file(s) -> {output_dir}", file=sys.stderr)

    mod = types.ModuleType("antenv.axon_hooks")
    mod.get_axon_ntff_profile_hook = lambda: _hook
    mod.set_axon_ntff_profile_hook = lambda h: None
    sys.modules["antenv.axon_hooks"] = mod


# ---------------------------------------------------------------------------
# host-side schedule
# ---------------------------------------------------------------------------
def build_schedule(x, edge_index, batch, num_graphs):
    N = x.shape[0]
    G = int(num_graphs)
    src = np.asarray(edge_index[0], dtype=np.int64)
    dst = np.asarray(edge_index[1], dtype=np.int64)
    batch = np.asarray(batch, dtype=np.int64)

    sizes = np.bincount(batch, minlength=G)
    gstart = np.zeros(G + 1, np.int64)
    np.cumsum(sizes, out=gstart[1:])

    first_idx = np.full(G, np.iinfo(np.int32).max, np.int64)
    np.minimum.at(first_idx, batch, np.arange(N))
    first_idx = np.clip(first_idx, 0, N - 1)

    deg = (np.bincount(dst, minlength=N) + 1).astype(np.float32)

    assert G % NCORES == 0, "graph count must divide core count"
    GPC = G // NCORES
    order = np.argsort(-sizes, kind="stable")
    g2core = np.zeros(G, np.int64)
    g2slot = np.zeros(G, np.int64)
    for i, g in enumerate(order):
        rnd, pos = divmod(i, NCORES)
        core = pos if rnd % 2 == 0 else NCORES - 1 - pos
        g2core[g] = core
        g2slot[g] = rnd

    S_GRAPH = max(512, int(np.ceil(sizes.max() / P)) * P)
    WPG = S_GRAPH // P
    NWIN = GPC * WPG

    core_graphs = []
    for c in range(NCORES):
        gs = [g for g in range(G) if g2core[g] == c]
        gs.sort(key=lambda g: g2slot[g])
        core_graphs.append(gs)
    real_c = np.array([sizes[core_graphs[c]].sum() for c in range(NCORES)])
    S_SHARD = int(np.ceil(real_c.max() / P)) * P
    NT = S_SHARD * NCORES
    HALF = NT // 2
    assert HALF % P == 0 and HALF < 32768, f"half table {HALF} must fit int16"

    shard_row = np.zeros(N, np.int64)
    pad_w = np.zeros(N, np.int64)
    pad_col = np.zeros(N, np.int64)
    node_core = g2core[batch]
    for c in range(NCORES):
        pos = 0
        for g in core_graphs[c]:
            n0, n1 = gstart[g], gstart[g + 1]
            cnt = n1 - n0
            ar = np.arange(cnt)
            shard_row[n0:n1] = c * S_SHARD + pos + ar
            slot = g2slot[g]
            pad_w[n0:n1] = slot * WPG + ar // P
            pad_col[n0:n1] = ar % P
            pos += cnt

    # edge + self-loop lists per core
    ecore = node_core[dst]
    t_rows, t_wins, t_cols = [], [], []
    for c in range(NCORES):
        m = ecore == c
        nm = node_core == c
        t_rows.append(np.concatenate([shard_row[src[m]], shard_row[nm]]))
        t_wins.append(np.concatenate([pad_w[dst[m]], pad_w[nm]]))
        t_cols.append(np.concatenate([pad_col[dst[m]], pad_col[nm]]))

    counts = np.zeros((NCORES, NWIN, 2), np.int64)
    for c in range(NCORES):
        h = (t_rows[c] >= HALF).astype(np.int64)
        np.add.at(counts, (c, t_wins[c], h), 1)
    cmax = counts.max(axis=0)
    chunks_wh = np.ceil(cmax / P).astype(np.int64)
    chunks_wh[:, 0] = np.maximum(chunks_wh[:, 0], 1)  # >=1 chunk so PSUM is written
    slots_wh = chunks_wh * P
    half_len = [int(slots_wh[:, h].sum()) for h in (0, 1)]
    TOTC = int(chunks_wh.sum())
    # per-half slot offset of each window's segment
    seg_off = np.zeros((NWIN, 2), np.int64)
    seg_off[1:, 0] = np.cumsum(slots_wh[:-1, 0])
    seg_off[1:, 1] = np.cumsum(slots_wh[:-1, 1])

    src16 = [[None, None] for _ in range(NCORES)]
    dstcol = np.full((NCORES, P, TOTC), 200.0, np.float32)
    for c in range(NCORES):
        rows, wins, cols = t_rows[c], t_wins[c], t_cols[c]
        h = (rows >= HALF).astype(np.int64)
        ci = 0
        for hh in (0, 1):
            stream = np.zeros(half_len[hh], np.int64)
            base = 0
            for w in range(NWIN):
                m = (wins == w) & (h == hh)
                k = int(m.sum())
                stream[base:base + k] = rows[m] - hh * HALF
                base += int(slots_wh[w, hh])
            src16[c][hh] = stream.astype(np.int16)
        # dstcol columns in window-major chunk order: for w: for hh: for k
        for w in range(NWIN):
            for hh in (0, 1):
                m = (wins == w) & (h == hh)
                cw = cols[m]
                for k in range(int(chunks_wh[w, hh])):
                    seg = cw[k * P:(k + 1) * P]
                    dstcol[c, :len(seg), ci] = seg
                    ci += 1
        assert ci == TOTC

    def wrap16(v):
        m = v.reshape(-1, 16).T  # [16, S]
        return np.ascontiguousarray(np.tile(m, (8, 1)).astype(np.int16))

    idx_w = [np.stack([wrap16(src16[c][hh]) for c in range(NCORES)]) for hh in (0, 1)]

    F = x.shape[1]
    x_core = np.zeros((NCORES, S_SHARD, F), np.float32)
    deg_padT = np.ones((NCORES, NWIN, P), np.float32)
    deg_shard = np.ones((NCORES, P, S_SHARD // P), np.float32)
    deg_pad = np.ones((NCORES, P, NWIN), np.float32)
    maskneg = np.full((NCORES, P, NWIN), -1e30, np.float32)
    rootsT = np.zeros((NCORES, F, GPC), np.float32)
    xf = np.asarray(x, dtype=np.float32)
    for c in range(NCORES):
        pos = 0
        for g in core_graphs[c]:
            n0, n1 = gstart[g], gstart[g + 1]
            cnt = n1 - n0
            x_core[c, pos:pos + cnt] = xf[n0:n1]
            pos += cnt
        nm = node_core == c
        sr = shard_row[nm] - c * S_SHARD
        d = deg[nm]
        deg_shard[c, sr % P, sr // P] = d
        deg_pad[c, pad_col[nm], pad_w[nm]] = d
        deg_padT[c, pad_w[nm], pad_col[nm]] = d
        maskneg[c, pad_col[nm], pad_w[nm]] = 0.0
        for j, g in enumerate(core_graphs[c]):
            rootsT[c, :, j] = xf[first_idx[g]]

    out_map = np.array([core_graphs[c] for c in range(NCORES)])

    return dict(
        G=G, GPC=GPC, F=F, S_GRAPH=S_GRAPH, WPG=WPG, NWIN=NWIN,
        S_SHARD=S_SHARD, NT=NT, HALF=HALF,
        chunks_wh=chunks_wh, slots_wh=slots_wh, seg_off=seg_off,
        half_len=half_len, TOTC=TOTC,
        idxA=idx_w[0], idxB=idx_w[1], dstcol=dstcol,
        x_core=x_core, deg_shard=deg_shard, deg_pad=deg_pad,
        deg_padT=deg_padT,
        maskneg=maskneg, rootsT=rootsT, out_map=out_map,
    )


# ---------------------------------------------------------------------------
# bass program
# ---------------------------------------------------------------------------
def build_program(s, NCLS, need_mask):
    import concourse.bass as bass
    import concourse.bacc as bacc
    import concourse.tile as tile
    import concourse.mybir as mybir

    f32 = mybir.dt.float32
    bf16 = mybir.dt.bfloat16
    i16 = mybir.dt.int16
    AF = mybir.ActivationFunctionType
    ALU = mybir.AluOpType
    AX = mybir.AxisListType

    F, GPC, NWIN, WPG = s["F"], s["GPC"], s["NWIN"], s["WPG"]
    S_SHARD, NT, HALF, TOTC = s["S_SHARD"], s["NT"], s["HALF"], s["TOTC"]
    NKT = F // P                 # k chunks for the 768-dim contraction
    NST = S_SHARD // P           # shard tiles
    chunks_wh = s["chunks_wh"]
    slots_wh = s["slots_wh"]
    seg_off = s["seg_off"]
    half_len = s["half_len"]

    nc = bacc.Bacc("TRN2", target_bir_lowering=False, debug=False,
                   enable_asserts=False, num_devices=NCORES,
                   num_swdge_queues=NQUEUES,
                   dynamic_dma_scratch_size=32768)

    OHG = 8  # chunks per batched one-hot build

    # inputs
    t_xT = nc.dram_tensor("xT_core", [F, S_SHARD], bf16, kind="ExternalInput")
    t_W1 = nc.dram_tensor("W1", [F, P], f32, kind="ExternalInput")
    t_W0 = nc.dram_tensor("W0", [F, P], f32, kind="ExternalInput")
    t_Wl1 = nc.dram_tensor("Wl1", [2 * P, P], f32, kind="ExternalInput")
    t_Wl2 = nc.dram_tensor("Wl2", [P, NCLS], f32, kind="ExternalInput")
    t_b1 = nc.dram_tensor("b1_rep", [P, P], f32, kind="ExternalInput")
    t_b0 = nc.dram_tensor("b0_rep", [P, P], f32, kind="ExternalInput")
    t_bl1 = nc.dram_tensor("bl1_rep", [P, P], f32, kind="ExternalInput")
    t_bl2 = nc.dram_tensor("bl2_rep", [P, NCLS], f32, kind="ExternalInput")
    t_degs = nc.dram_tensor("deg_shard", [P, NST], f32, kind="ExternalInput")
    t_degp = nc.dram_tensor("deg_pad", [P, NWIN], f32, kind="ExternalInput")
    t_degpT = nc.dram_tensor("deg_padT", [1, NWIN * P], f32, kind="ExternalInput")
    t_mask = nc.dram_tensor("maskneg", [P, NWIN], f32, kind="ExternalInput")
    t_roots = nc.dram_tensor("rootsT", [F, GPC], f32, kind="ExternalInput")
    t_idxA = nc.dram_tensor("idxA", [P, half_len[0] // 16], i16, kind="ExternalInput")
    t_idxB = nc.dram_tensor("idxB", [P, half_len[1] // 16], i16, kind="ExternalInput")
    t_dcol = nc.dram_tensor("dstcol", [P, TOTC], f32, kind="ExternalInput")
    t_iota = nc.dram_tensor("iota_row", [P, P], f32, kind="ExternalInput")
    t_iota8 = nc.dram_tensor("iota8", [P, OHG * P], f32, kind="ExternalInput")
    t_ident = nc.dram_tensor("identity", [P, P], f32, kind="ExternalInput")
    t_y = nc.dram_tensor("y", [GPC, NCLS], f32, kind="ExternalOutput")

    with tile.TileContext(nc) as tc:
        with tc.tile_pool(name="const", bufs=1) as cst, \
             tc.tile_pool(name="small", bufs=3) as sm, \
             tc.tile_pool(name="psA", bufs=2, space="PSUM") as psA, \
             tc.tile_pool(name="psB", bufs=2, space="PSUM") as psB, \
             tc.tile_pool(name="dram", bufs=1, space="DRAM") as dram:

            # ---- constants in SBUF ----
            ident_f = cst.tile([P, P], f32)
            nc.sync.dma_start(ident_f[:], t_ident[:])
            ident_bf = cst.tile([P, P], bf16)
            nc.vector.tensor_copy(ident_bf[:], ident_f[:])
            iota_sb = cst.tile([P, P], f32)
            nc.sync.dma_start(iota_sb[:], t_iota[:])
            iota8_sb = cst.tile([P, OHG * P], f32)
            nc.sync.dma_start(iota8_sb[:], t_iota8[:])
            b1_sb = cst.tile([P, P], f32)
            nc.sync.dma_start(b1_sb[:], t_b1[:])
            b0_sb = cst.tile([P, P], f32)
            nc.sync.dma_start(b0_sb[:], t_b0[:])
            bl1_sb = cst.tile([P, P], f32)
            nc.sync.dma_start(bl1_sb[:], t_bl1[:])
            bl2_sb = cst.tile([P, NCLS], f32)
            nc.sync.dma_start(bl2_sb[:], t_bl2[:])
            dcol_sb = cst.tile([P, TOTC], f32)
            nc.sync.dma_start(dcol_sb[:], t_dcol[:])
            idxA_sb = cst.tile([P, half_len[0] // 16], i16)
            nc.sync.dma_start(idxA_sb[:], t_idxA[:])
            idxB_sb = cst.tile([P, half_len[1] // 16], i16)
            nc.sync.dma_start(idxB_sb[:], t_idxB[:])

            # dinv arrays: 1/sqrt(deg) = sqrt(1/deg)
            degs_sb = cst.tile([P, NST], f32)
            nc.sync.dma_start(degs_sb[:], t_degs[:])
            dinvs_sb = cst.tile([P, NST], f32)
            nc.vector.reciprocal(dinvs_sb[:], degs_sb[:])
            nc.scalar.activation(dinvs_sb[:], dinvs_sb[:], AF.Sqrt)
            degp_sb = cst.tile([P, NWIN], f32)
            nc.sync.dma_start(degp_sb[:], t_degp[:])
            dinvp_sb = cst.tile([P, NWIN], f32)
            nc.vector.reciprocal(dinvp_sb[:], degp_sb[:])
            nc.scalar.activation(dinvp_sb[:], dinvp_sb[:], AF.Sqrt)
            mask_sb = cst.tile([P, NWIN], f32)
            nc.sync.dma_start(mask_sb[:], t_mask[:])
            # sqrt(deg) per (window, col) transposed + b1 row, both bf16, for
            # the K=1 bias matmul injecting b1*sqrt(deg) into each window PSUM
            degpT_f = cst.tile([1, NWIN * P], f32)
            nc.sync.dma_start(degpT_f[:], t_degpT[:])
            nc.scalar.activation(degpT_f[:], degpT_f[:], AF.Sqrt)
            sdegT_bf = cst.tile([1, NWIN * P], bf16)
            nc.vector.tensor_copy(sdegT_bf[:], degpT_f[:])
            b1row_bf = cst.tile([1, P], bf16)
            nc.vector.tensor_copy(b1row_bf[:], b1_sb[:1, :])

            # W1 as bf16 k-chunk tiles
            W1_bf = []
            for kc in range(NKT):
                wt = cst.tile([P, P], f32, tag="w1f")
                nc.sync.dma_start(wt[:], t_W1[kc * P:(kc + 1) * P, :])
                wb = cst.tile([P, P], bf16, tag=f"w1b{kc}")
                nc.vector.tensor_copy(wb[:], wt[:])
                W1_bf.append(wb)

            # ---- phase 1: h' shard (x pre-transposed on host) ----
            h_in = dram.tile([S_SHARD, P], bf16)
            h_full = dram.tile([NT, P], bf16, addr_space="Shared")
            with tc.tile_pool(name="xstr", bufs=1) as xsp:
                xbs = []
                for kc in range(NKT):
                    xb = xsp.tile([P, S_SHARD], bf16, tag=f"xb{kc}")
                    nc.sync.dma_start(xb[:], t_xT[kc * P:(kc + 1) * P, :])
                    xbs.append(xb)
                for t in range(NST):
                    hps = psB.tile([P, P], f32, tag="acc")
                    for kc in range(NKT):
                        nc.tensor.matmul(hps[:], lhsT=xbs[kc][:, t * P:(t + 1) * P],
                                         rhs=W1_bf[kc][:],
                                         start=(kc == 0), stop=(kc == NKT - 1))
                    hp = sm.tile([P, P], bf16, tag="hp")
                    nc.vector.tensor_scalar(out=hp[:], in0=hps[:],
                                            scalar1=dinvs_sb[:, t:t + 1], scalar2=None,
                                            op0=ALU.mult)
                    nc.sync.dma_start(h_in[t * P:(t + 1) * P, :], hp[:])

            # ---- allgather ----
            nc.gpsimd.collective_compute(
                "AllGather", ALU.bypass,
                replica_groups=[list(range(NCORES))],
                ins=[h_in.opt()],
                outs=[h_full.opt()],
            )

            # phase-2 pools open after the x-streaming pool is released
            es = contextlib.ExitStack()
            gp = es.enter_context(tc.tile_pool(name="gat", bufs=1))
            ohp = es.enter_context(tc.tile_pool(name="ohp", bufs=1))

            # ---- phase 2: edge aggregation ----
            # gather ops per half: list of (chunk_base_slot, nchunks)
            def half_ops(L):
                ops = []
                base = 0
                while base < L:
                    n = min(OPCHUNKS * P, L - base)
                    ops.append((base, n))
                    base += n
                return ops

            opsA = half_ops(half_len[0])
            opsB = half_ops(half_len[1])
            gtiles = {0: {}, 1: {}}
            idx_sb = {0: idxA_sb, 1: idxB_sb}
            tabs = {0: h_full[0:HALF, :], 1: h_full[HALF:NT, :]}

            gq_counter = [0]

            def issue_gather(hh, opi, base, nsl):
                g = gp.tile([P, OPCHUNKS * P], bf16, tag=f"g{hh}", bufs=GBUFS)
                nc.gpsimd.dma_gather(
                    g[:, :nsl].rearrange("p (c f) -> p c f", f=P),
                    tabs[hh],
                    idx_sb[hh][:, base // 16: (base + nsl) // 16],
                    nsl, nsl, P,
                    queue_num=gq_counter[0] % NQUEUES,
                )
                gq_counter[0] += 1
                gtiles[hh][opi] = g

            for opi, (base, nsl) in enumerate(opsA):
                issue_gather(0, opi, base, nsl)
            for opi, (base, nsl) in enumerate(opsB):
                issue_gather(1, opi, base, nsl)

            # batched one-hot builds: one DVE op per OHG chunks
            oh_tiles = {}

            def onehot_group(g0):
                n = min(OHG, TOTC - g0)
                oh = ohp.tile([P, OHG * P], bf16, tag="oh", bufs=8)
                nc.vector.tensor_tensor(
                    out=oh[:, :n * P].rearrange("p (c f) -> p c f", f=P),
                    in0=iota8_sb[:, :n * P].rearrange("p (c f) -> p c f", f=P),
                    in1=dcol_sb[:, g0:g0 + n].to_broadcast([P, n, P]),
                    op=ALU.is_equal)
                oh_tiles[g0] = oh

            winmax_sb = cst.tile([P, NWIN], f32)
            TRW = 4  # windows per transpose/reduce batch
            ci = 0
            for w in range(NWIN):
                if w % TRW == 0:
                    tr = psA.tile([P, TRW * P], bf16, tag="tp")
                agg = psB.tile([P, P], f32, tag="acc")
                # bias chunk: agg += sqrt(deg)[col] * b1[f]
                nc.tensor.matmul(agg[:], lhsT=sdegT_bf[:1, w * P:(w + 1) * P],
                                 rhs=b1row_bf[:1, :], start=True, stop=False)
                nch = int(chunks_wh[w, 0] + chunks_wh[w, 1])
                j = 0
                for hh in (0, 1):
                    for k in range(int(chunks_wh[w, hh])):
                        slot = int(seg_off[w, hh]) + k * P
                        opi, off = divmod(slot, OPCHUNKS * P)
                        g = gtiles[hh][opi]
                        if ci % OHG == 0:
                            onehot_group(ci)
                        oh = oh_tiles[(ci // OHG) * OHG]
                        nc.tensor.matmul(agg[:], lhsT=oh[:, (ci % OHG) * P:(ci % OHG + 1) * P],
                                         rhs=g[:, off:off + P],
                                         start=False, stop=(j == nch - 1))
                        ci += 1
                        j += 1
                # hx = relu(dinv * (agg + sqrt(deg)*b1)) = relu(dinv*agg + b1)
                hx = sm.tile([P, P], bf16, tag="hx")
                nc.scalar.activation(hx[:], agg[:], AF.Relu,
                                     scale=dinvp_sb[:, w:w + 1])
                if need_mask:
                    nc.vector.tensor_scalar(out=hx[:], in0=hx[:],
                                            scalar1=mask_sb[:, w:w + 1], scalar2=None,
                                            op0=ALU.add)
                nc.tensor.transpose(tr[:, (w % TRW) * P:(w % TRW + 1) * P],
                                    hx[:], ident_bf[:])
                if w % TRW == TRW - 1:
                    nc.vector.reduce_max(
                        out=winmax_sb[:, w - TRW + 1:w + 1],
                        in_=tr[:].rearrange("p (c f) -> p c f", f=P), axis=AX.X)
            assert ci == TOTC

            # ---- pooling: per-graph max over its windows ----
            pooled_sb = cst.tile([P, GPC], f32)
            for g in range(GPC):
                nc.vector.reduce_max(out=pooled_sb[:, g:g + 1],
                                     in_=winmax_sb[:, g * WPG:(g + 1) * WPG], axis=AX.X)

            # ---- news = relu(x_root @ W0 + b0) ----
            nps = psB.tile([GPC, P], f32, tag="acc")
            for kc in range(NKT):
                rt = sm.tile([P, GPC], f32, tag="rt")
                nc.sync.dma_start(rt[:], t_roots[kc * P:(kc + 1) * P, :])
                w0t = sm.tile([P, P], f32, tag="w0t")
                nc.sync.dma_start(w0t[:], t_W0[kc * P:(kc + 1) * P, :])
                nc.tensor.matmul(nps[:], lhsT=rt[:], rhs=w0t[:],
                                 start=(kc == 0), stop=(kc == NKT - 1))
            news = sm.tile([GPC, P], f32, tag="news")
            nc.vector.tensor_add(news[:], nps[:], b0_sb[:GPC, :])
            nc.scalar.activation(news[:], news[:], AF.Relu)
            ntr = psA.tile([P, GPC], f32, tag="tp")
            nc.tensor.transpose(ntr[:], news[:], ident_f[:GPC, :GPC])
            newsT = sm.tile([P, GPC], f32, tag="newsT")
            nc.vector.tensor_copy(newsT[:], ntr[:])

            # ---- z = relu([pooled|news] @ Wl1 + bl1) ----
            wl1a = sm.tile([P, P], f32, tag="wl1a")
            nc.sync.dma_start(wl1a[:], t_Wl1[0:P, :])
            wl1b = sm.tile([P, P], f32, tag="wl1b")
            nc.sync.dma_start(wl1b[:], t_Wl1[P:2 * P, :])
            zps = psB.tile([GPC, P], f32, tag="acc")
            nc.tensor.matmul(zps[:], lhsT=pooled_sb[:], rhs=wl1a[:], start=True, stop=False)
            nc.tensor.matmul(zps[:], lhsT=newsT[:], rhs=wl1b[:], start=False, stop=True)
            z2 = sm.tile([GPC, P], f32, tag="z2")
            nc.vector.tensor_add(z2[:], zps[:], bl1_sb[:GPC, :])
            nc.scalar.activation(z2[:], z2[:], AF.Relu)
            ztr = psA.tile([P, GPC], f32, tag="tp")
            nc.tensor.transpose(ztr[:], z2[:], ident_f[:GPC, :GPC])
            z2T = sm.tile([P, GPC], f32, tag="z2T")
            nc.vector.tensor_copy(z2T[:], ztr[:])

            # ---- logits + log_softmax ----
            wl2 = sm.tile([P, NCLS], f32, tag="wl2")
            nc.sync.dma_start(wl2[:], t_Wl2[:])
            lps = psB.tile([GPC, NCLS], f32, tag="acc")
            nc.tensor.matmul(lps[:], lhsT=z2T[:], rhs=wl2[:], start=True, stop=True)
            lg = sm.tile([GPC, NCLS], f32, tag="lg")
            nc.vector.tensor_add(lg[:], lps[:], bl2_sb[:GPC, :])
            mx = sm.tile([GPC, 1], f32, tag="mx")
            nc.vector.reduce_max(out=mx[:], in_=lg[:], axis=AX.X)
            tt = sm.tile([GPC, NCLS], f32, tag="tt")
            nc.vector.tensor_scalar(out=tt[:], in0=lg[:], scalar1=mx[:],
                                    scalar2=None, op0=ALU.subtract)
            ee = sm.tile([GPC, NCLS], f32, tag="ee")
            nc.scalar.activation(ee[:], tt[:], AF.Exp)
            ss = sm.tile([GPC, 1], f32, tag="ss")
            nc.vector.reduce_sum(out=ss[:], in_=ee[:], axis=AX.X)
            ls = sm.tile([GPC, 1], f32, tag="ls")
            nc.scalar.activation(ls[:], ss[:], AF.Ln)
            yy = sm.tile([GPC, NCLS], f32, tag="yy")
            nc.vector.tensor_scalar(out=yy[:], in0=tt[:], scalar1=ls[:],
                                    scalar2=None, op0=ALU.subtract)
            nc.sync.dma_start(t_y[:], yy[:])
            es.close()

    nc.compile()
    return nc


# ---------------------------------------------------------------------------
# entry point
# ---------------------------------------------------------------------------
def kernel(**inputs) -> np.ndarray:
    global LAST_EXEC_NS
    _install_ntff_hook()
    from concourse import bass_utils
    from concourse.bass_interp import get_hw_module

    x = np.asarray(inputs["x"], dtype=np.float32)
    ei = np.asarray(inputs["edge_index"])
    batch = np.asarray(inputs["batch"])
    G = int(np.asarray(inputs["num_graphs"]))
    W1 = np.asarray(inputs["W1"], dtype=np.float32)
    b1 = np.asarray(inputs["b1"], dtype=np.float32)
    W0 = np.asarray(inputs["W0"], dtype=np.float32)
    b0 = np.asarray(inputs["b0"], dtype=np.float32)
    Wl1 = np.asarray(inputs["Wl1"], dtype=np.float32)
    bl1 = np.asarray(inputs["bl1"], dtype=np.float32)
    Wl2 = np.asarray(inputs["Wl2"], dtype=np.float32)
    bl2 = np.asarray(inputs["bl2"], dtype=np.float32)
    NCLS = Wl2.shape[1]

    s = build_schedule(x, ei, batch, G)
    need_mask = bool((b1 > 0).any())
    nc = build_program(s, NCLS, need_mask)

    rep = lambda v, n: np.ascontiguousarray(np.tile(v[None, :], (n, 1)).astype(np.float32))
    iota_row = rep(np.arange(P, dtype=np.float32), P)
    iota8 = np.ascontiguousarray(np.tile(iota_row, (1, 8)))
    ident = np.eye(P, dtype=np.float32)

    in_maps = []
    for c in range(NCORES):
        in_maps.append({
            "xT_core": np.ascontiguousarray(s["x_core"][c].T).astype(ml_dtypes.bfloat16),
            "W1": W1, "W0": W0, "Wl1": Wl1, "Wl2": Wl2,
            "b1_rep": rep(b1, P), "b0_rep": rep(b0, P),
            "bl1_rep": rep(bl1, P), "bl2_rep": rep(bl2, P),
            "deg_shard": s["deg_shard"][c], "deg_pad": s["deg_pad"][c],
            "deg_padT": np.ascontiguousarray(s["deg_padT"][c].reshape(1, -1)),
            "maskneg": s["maskneg"][c], "rootsT": s["rootsT"][c],
            "idxA": s["idxA"][c], "idxB": s["idxB"][c],
            "dstcol": s["dstcol"][c],
            "iota_row": iota_row, "iota8": iota8, "identity": ident,
        })

    nc.m = get_hw_module(nc.m)
    res = bass_utils.run_bass_kernel_spmd(
        nc, in_maps, core_ids=list(range(NCORES)), trace=TRACE)
    LAST_EXEC_NS = res.exec_time_ns

    out = np.zeros((G, NCLS), np.float32)
    for c in range(NCORES):
        out[s["out_map"][c]] = res.results[c]["y"]
    return out



# revision 22
# speedup vs baseline: 2.7831x; 1.0830x over previous
"""TRN2 Bass kernel for nn_GCNModel: GCNConv + global max pool + root MLP head.

Strategy (8 NeuronCores, SPMD):
  - Graphs are assigned to cores (16 each, snake order by size for balance).
    Each core owns its graphs' nodes (contiguous ranges of the sorted batch).
  - h' = (x @ W1) * rsqrt(deg) computed shard-local in bf16 (PE, fp32 PSUM),
    then AllGather'd into a replicated row table [NT, 128] bf16 in DRAM.
  - Edge aggregation per core: edges (+ self loops) bucketed by
    (dst 128-node window, src table half), gathered row-wise from the table
    via gpsimd dma_gather (int16 half-local indices), then scattered into
    per-window PSUM accumulators with one-hot matmuls (exact, fp32 accum).
  - hx = relu(dinv * agg + b1) masked with -1e30 on pad rows; per-window
    column max after a PE transpose; per-graph max over its windows.
  - Head: news = relu(x_root @ W0 + b0); z = [pooled|news] @ Wl1 -> relu ->
    @ Wl2 -> log_softmax. All fp32.

The chunk schedule is made SPMD-uniform by padding per-(window,half) chunk
counts to the max over cores; pad lanes use dst column 200 (one-hot miss) so
they contribute exactly zero.
"""
import sys
import types
import contextlib
import ctypes

import numpy as np
import ml_dtypes

NCORES = 8
P = 128
OPCHUNKS = 8           # chunks (of 128 slots) per dma_gather op (1024 descs fits the ring)
NQUEUES = 4            # SWDGE queues; gathers round-robin so transfers overlap
GBUFS = 6              # in-flight gather buffers per table half
TRACE = False          # set by test.py for profiling
LAST_EXEC_NS = None


# ---------------------------------------------------------------------------
# axon NTFF profile hook (the image's antenv lacks axon_hooks)
# ---------------------------------------------------------------------------
def _install_ntff_hook():
    if "antenv.axon_hooks" in sys.modules:
        return
    try:
        lib = ctypes.CDLL("/opt/axon/libaxon_pjrt.so")
        if not hasattr(lib, "axon_start_nrt_profile"):
            return
    except OSError:
        return
    lib.axon_start_nrt_profile.argtypes = [ctypes.POINTER(ctypes.c_int64), ctypes.c_size_t]
    lib.axon_start_nrt_profile.restype = ctypes.c_int64
    lib.axon_stop_nrt_profile.argtypes = [ctypes.c_char_p]
    lib.axon_stop_nrt_profile.restype = ctypes.c_int64

    @contextlib.contextmanager
    def _hook(output_dir, device_ids):
        import jax
        jax.devices()
        if device_ids:
            ids = (ctypes.c_int64 * len(device_ids))(*device_ids)
            rc = lib.axon_start_nrt_profile(ids, len(device_ids))
        else:
            rc = lib.axon_start_nrt_profile(None, 0)
        if rc != 0:
            raise RuntimeError(f"axon_start_nrt_profile rc={rc}")
        try:
            yield
        finally:
            n = lib.axon_stop_nrt_profile(str(output_dir).encode())
            print(f"ntff profile: {n} file(s) -> {output_dir}", file=sys.stderr)

    mod = types.ModuleType("antenv.axon_hooks")
    mod.get_axon_ntff_profile_hook = lambda: _hook
    mod.set_axon_ntff_profile_hook = lambda h: None
    sys.modules["antenv.axon_hooks"] = mod


# ---------------------------------------------------------------------------
# host-side schedule
# ---------------------------------------------------------------------------
def build_schedule(x, edge_index, batch, num_graphs):
    N = x.shape[0]
    G = int(num_graphs)
    src = np.asarray(edge_index[0], dtype=np.int64)
    dst = np.asarray(edge_index[1], dtype=np.int64)
    batch = np.asarray(batch, dtype=np.int64)

    sizes = np.bincount(batch, minlength=G)
    gstart = np.zeros(G + 1, np.int64)
    np.cumsum(sizes, out=gstart[1:])

    first_idx = np.full(G, np.iinfo(np.int32).max, np.int64)
    np.minimum.at(first_idx, batch, np.arange(N))
    first_idx = np.clip(first_idx, 0, N - 1)

    deg = (np.bincount(dst, minlength=N) + 1).astype(np.float32)

    assert G % NCORES == 0, "graph count must divide core count"
    GPC = G // NCORES
    order = np.argsort(-sizes, kind="stable")
    g2core = np.zeros(G, np.int64)
    g2slot = np.zeros(G, np.int64)
    for i, g in enumerate(order):
        rnd, pos = divmod(i, NCORES)
        core = pos if rnd % 2 == 0 else NCORES - 1 - pos
        g2core[g] = core
        g2slot[g] = rnd

    S_GRAPH = max(512, int(np.ceil(sizes.max() / P)) * P)
    WPG = S_GRAPH // P
    NWIN = GPC * WPG

    core_graphs = []
    for c in range(NCORES):
        gs = [g for g in range(G) if g2core[g] == c]
        gs.sort(key=lambda g: g2slot[g])
        core_graphs.append(gs)
    real_c = np.array([sizes[core_graphs[c]].sum() for c in range(NCORES)])
    S_SHARD = int(np.ceil(real_c.max() / P)) * P
    NT = S_SHARD * NCORES
    HALF = NT // 2
    assert HALF % P == 0 and HALF < 32768, f"half table {HALF} must fit int16"

    shard_row = np.zeros(N, np.int64)
    pad_w = np.zeros(N, np.int64)
    pad_col = np.zeros(N, np.int64)
    node_core = g2core[batch]
    for c in range(NCORES):
        pos = 0
        for g in core_graphs[c]:
            n0, n1 = gstart[g], gstart[g + 1]
            cnt = n1 - n0
            ar = np.arange(cnt)
            shard_row[n0:n1] = c * S_SHARD + pos + ar
            slot = g2slot[g]
            pad_w[n0:n1] = slot * WPG + ar // P
            pad_col[n0:n1] = ar % P
            pos += cnt

    # edge + self-loop lists per core
    ecore = node_core[dst]
    t_rows, t_wins, t_cols = [], [], []
    for c in range(NCORES):
        m = ecore == c
        nm = node_core == c
        t_rows.append(np.concatenate([shard_row[src[m]], shard_row[nm]]))
        t_wins.append(np.concatenate([pad_w[dst[m]], pad_w[nm]]))
        t_cols.append(np.concatenate([pad_col[dst[m]], pad_col[nm]]))

    counts = np.zeros((NCORES, NWIN, 2), np.int64)
    for c in range(NCORES):
        h = (t_rows[c] >= HALF).astype(np.int64)
        np.add.at(counts, (c, t_wins[c], h), 1)
    # tight segments: one per (window, half), max over cores, no 128-rounding
    seg_len = counts.max(axis=0)
    seg_len[:, 0] = np.maximum(seg_len[:, 0], 1)  # >=1 part so PSUM is written
    seg_off = np.zeros((NWIN, 2), np.int64)
    seg_off[1:, 0] = np.cumsum(seg_len[:-1, 0])
    seg_off[1:, 1] = np.cumsum(seg_len[:-1, 1])
    half_real = [int(seg_len[:, hh].sum()) for hh in (0, 1)]
    half_len = [-(-r // P) * P for r in half_real]
    TOTC_h = [half_len[hh] // P for hh in (0, 1)]
    TOTC = TOTC_h[0] + TOTC_h[1]

    # parts: per window (in order), the chunk ranges its segments occupy.
    # Each part scatters a full 128-slot chunk with cols outside [a,b)
    # masked to 200 (one-hot miss), so straddling chunks need no partition
    # slicing on the PE.
    parts = []  # (w, hh, lc, a, b)
    for w in range(NWIN):
        for hh in (0, 1):
            s0 = int(seg_off[w, hh])
            s1 = s0 + int(seg_len[w, hh])
            for lc in range(s0 // P, (s1 + P - 1) // P):
                a = max(0, s0 - lc * P)
                b = min(P, s1 - lc * P)
                parts.append((w, hh, lc, a, b))
    NPARTS = len(parts)

    src16 = [[None, None] for _ in range(NCORES)]
    dstcol = np.full((NCORES, P, NPARTS), 200.0, np.float32)
    for c in range(NCORES):
        rows, wins, cols = t_rows[c], t_wins[c], t_cols[c]
        h = (rows >= HALF).astype(np.int64)
        colstrs = {}
        for hh in (0, 1):
            stream = np.zeros(half_len[hh], np.int64)
            colstr = np.full(half_len[hh], 200.0, np.float32)
            for w in range(NWIN):
                m = (wins == w) & (h == hh)
                k = int(m.sum())
                base = int(seg_off[w, hh])
                srt = np.argsort(rows[m], kind="stable")
                stream[base:base + k] = rows[m][srt] - hh * HALF
                colstr[base:base + k] = cols[m][srt]
            src16[c][hh] = stream.astype(np.int16)
            colstrs[hh] = colstr
        for pi, (w, hh, lc, a, b) in enumerate(parts):
            dstcol[c, a:b, pi] = colstrs[hh][lc * P + a: lc * P + b]

    def wrap16(v):
        m = v.reshape(-1, 16).T  # [16, S]
        return np.ascontiguousarray(np.tile(m, (8, 1)).astype(np.int16))

    idx_w = [np.stack([wrap16(src16[c][hh]) for c in range(NCORES)]) for hh in (0, 1)]

    F = x.shape[1]
    x_core = np.zeros((NCORES, S_SHARD, F), np.float32)
    deg_padT = np.ones((NCORES, NWIN, P), np.float32)
    deg_shard = np.ones((NCORES, P, S_SHARD // P), np.float32)
    deg_pad = np.ones((NCORES, P, NWIN), np.float32)
    maskneg = np.full((NCORES, P, NWIN), -1e30, np.float32)
    rootsT = np.zeros((NCORES, F, GPC), np.float32)
    xf = np.asarray(x, dtype=np.float32)
    for c in range(NCORES):
        pos = 0
        for g in core_graphs[c]:
            n0, n1 = gstart[g], gstart[g + 1]
            cnt = n1 - n0
            x_core[c, pos:pos + cnt] = xf[n0:n1]
            pos += cnt
        nm = node_core == c
        sr = shard_row[nm] - c * S_SHARD
        d = deg[nm]
        deg_shard[c, sr % P, sr // P] = d
        deg_pad[c, pad_col[nm], pad_w[nm]] = d
        deg_padT[c, pad_w[nm], pad_col[nm]] = d
        maskneg[c, pad_col[nm], pad_w[nm]] = 0.0
        for j, g in enumerate(core_graphs[c]):
            rootsT[c, :, j] = xf[first_idx[g]]

    out_map = np.array([core_graphs[c] for c in range(NCORES)])

    return dict(
        G=G, GPC=GPC, F=F, S_GRAPH=S_GRAPH, WPG=WPG, NWIN=NWIN,
        S_SHARD=S_SHARD, NT=NT, HALF=HALF,
        seg_len=seg_len, seg_off=seg_off, parts=parts, NPARTS=NPARTS,
        half_len=half_len, TOTC=TOTC, TOTC_h=TOTC_h,
        idxA=idx_w[0], idxB=idx_w[1], dstcol=dstcol,
        x_core=x_core, deg_shard=deg_shard, deg_pad=deg_pad,
        deg_padT=deg_padT,
        maskneg=maskneg, rootsT=rootsT, out_map=out_map,
    )


# ---------------------------------------------------------------------------
# bass program
# ---------------------------------------------------------------------------
def build_program(s, NCLS, need_mask, need_bias):
    import concourse.bass as bass
    import concourse.bacc as bacc
    import concourse.tile as tile
    import concourse.mybir as mybir

    f32 = mybir.dt.float32
    bf16 = mybir.dt.bfloat16
    i16 = mybir.dt.int16
    AF = mybir.ActivationFunctionType
    ALU = mybir.AluOpType
    AX = mybir.AxisListType

    F, GPC, NWIN, WPG = s["F"], s["GPC"], s["NWIN"], s["WPG"]
    S_SHARD, NT, HALF, TOTC = s["S_SHARD"], s["NT"], s["HALF"], s["TOTC"]
    NKT = F // P                 # k chunks for the 768-dim contraction
    NST = S_SHARD // P           # shard tiles
    seg_len = s["seg_len"]
    seg_off = s["seg_off"]
    half_len = s["half_len"]
    TOTC_h = s["TOTC_h"]
    parts_tab = s["parts"]
    NPARTS = s["NPARTS"]

    nc = bacc.Bacc("TRN2", target_bir_lowering=False, debug=False,
                   enable_asserts=False, num_devices=NCORES,
                   num_swdge_queues=NQUEUES,
                   dynamic_dma_scratch_size=32768)

    OHG = 8  # chunks per batched one-hot build

    # inputs
    t_xT = nc.dram_tensor("xT_core", [F, S_SHARD], bf16, kind="ExternalInput")
    t_W1 = nc.dram_tensor("W1", [F, P], f32, kind="ExternalInput")
    t_W0 = nc.dram_tensor("W0", [F, P], f32, kind="ExternalInput")
    t_Wl1 = nc.dram_tensor("Wl1", [2 * P, P], f32, kind="ExternalInput")
    t_Wl2 = nc.dram_tensor("Wl2", [P, NCLS], f32, kind="ExternalInput")
    t_b1 = nc.dram_tensor("b1_rep", [P, P], f32, kind="ExternalInput")
    t_b0 = nc.dram_tensor("b0_rep", [P, P], f32, kind="ExternalInput")
    t_bl1 = nc.dram_tensor("bl1_rep", [P, P], f32, kind="ExternalInput")
    t_bl2 = nc.dram_tensor("bl2_rep", [P, NCLS], f32, kind="ExternalInput")
    t_degs = nc.dram_tensor("deg_shard", [P, NST], f32, kind="ExternalInput")
    t_degp = nc.dram_tensor("deg_pad", [P, NWIN], f32, kind="ExternalInput")
    t_degpT = nc.dram_tensor("deg_padT", [1, NWIN * P], f32, kind="ExternalInput")
    t_mask = nc.dram_tensor("maskneg", [P, NWIN], f32, kind="ExternalInput")
    t_roots = nc.dram_tensor("rootsT", [F, GPC], f32, kind="ExternalInput")
    t_idxA = nc.dram_tensor("idxA", [P, half_len[0] // 16], i16, kind="ExternalInput")
    t_idxB = nc.dram_tensor("idxB", [P, half_len[1] // 16], i16, kind="ExternalInput")
    t_dcol = nc.dram_tensor("dstcol", [P, NPARTS], f32, kind="ExternalInput")
    t_iota = nc.dram_tensor("iota_row", [P, P], f32, kind="ExternalInput")
    t_iota8 = nc.dram_tensor("iota8", [P, OHG * P], f32, kind="ExternalInput")
    t_ident = nc.dram_tensor("identity", [P, P], f32, kind="ExternalInput")
    t_y = nc.dram_tensor("y", [GPC, NCLS], f32, kind="ExternalOutput")

    with tile.TileContext(nc) as tc:
        with tc.tile_pool(name="const", bufs=1) as cst, \
             tc.tile_pool(name="small", bufs=3) as sm, \
             tc.tile_pool(name="psA", bufs=2, space="PSUM") as psA, \
             tc.tile_pool(name="psB", bufs=2, space="PSUM") as psB, \
             tc.tile_pool(name="dram", bufs=1, space="DRAM") as dram:

            # ---- constants in SBUF ----
            ident_f = cst.tile([P, P], f32)
            nc.sync.dma_start(ident_f[:], t_ident[:])
            ident_bf = cst.tile([P, P], bf16)
            nc.vector.tensor_copy(ident_bf[:], ident_f[:])
            iota_sb = cst.tile([P, P], f32)
            nc.sync.dma_start(iota_sb[:], t_iota[:])
            iota8_sb = cst.tile([P, OHG * P], f32)
            nc.sync.dma_start(iota8_sb[:], t_iota8[:])
            b1_sb = cst.tile([P, P], f32)
            nc.sync.dma_start(b1_sb[:], t_b1[:])
            b0_sb = cst.tile([P, P], f32)
            nc.sync.dma_start(b0_sb[:], t_b0[:])
            bl1_sb = cst.tile([P, P], f32)
            nc.sync.dma_start(bl1_sb[:], t_bl1[:])
            bl2_sb = cst.tile([P, NCLS], f32)
            nc.sync.dma_start(bl2_sb[:], t_bl2[:])
            dcol_sb = cst.tile([P, NPARTS], f32)
            nc.sync.dma_start(dcol_sb[:], t_dcol[:])
            idxA_sb = cst.tile([P, half_len[0] // 16], i16)
            nc.sync.dma_start(idxA_sb[:], t_idxA[:])
            idxB_sb = cst.tile([P, half_len[1] // 16], i16)
            nc.sync.dma_start(idxB_sb[:], t_idxB[:])

            # dinv arrays: 1/sqrt(deg) = sqrt(1/deg)
            degs_sb = cst.tile([P, NST], f32)
            nc.sync.dma_start(degs_sb[:], t_degs[:])
            dinvs_sb = cst.tile([P, NST], f32)
            nc.vector.reciprocal(dinvs_sb[:], degs_sb[:])
            nc.scalar.activation(dinvs_sb[:], dinvs_sb[:], AF.Sqrt)
            degp_sb = cst.tile([P, NWIN], f32)
            nc.sync.dma_start(degp_sb[:], t_degp[:])
            dinvp_sb = cst.tile([P, NWIN], f32)
            nc.vector.reciprocal(dinvp_sb[:], degp_sb[:])
            nc.scalar.activation(dinvp_sb[:], dinvp_sb[:], AF.Sqrt)
            mask_sb = cst.tile([P, NWIN], f32)
            nc.sync.dma_start(mask_sb[:], t_mask[:])
            # sqrt(deg) per (window, col) transposed + b1 row, both bf16, for
            # the K=1 bias matmul injecting b1*sqrt(deg) into each window PSUM
            degpT_f = cst.tile([1, NWIN * P], f32)
            nc.sync.dma_start(degpT_f[:], t_degpT[:])
            nc.scalar.activation(degpT_f[:], degpT_f[:], AF.Sqrt)
            sdegT_bf = cst.tile([1, NWIN * P], bf16)
            nc.vector.tensor_copy(sdegT_bf[:], degpT_f[:])
            b1row_bf = cst.tile([1, P], bf16)
            nc.vector.tensor_copy(b1row_bf[:], b1_sb[:1, :])

            # W1 as bf16 k-chunk tiles
            W1_bf = []
            for kc in range(NKT):
                wt = cst.tile([P, P], f32, tag="w1f")
                nc.sync.dma_start(wt[:], t_W1[kc * P:(kc + 1) * P, :])
                wb = cst.tile([P, P], bf16, tag=f"w1b{kc}")
                nc.vector.tensor_copy(wb[:], wt[:])
                W1_bf.append(wb)

            # ---- phase 1: h' shard (x pre-transposed on host) ----
            h_in = dram.tile([S_SHARD, P], bf16)
            h_full = dram.tile([NT, P], bf16, addr_space="Shared")
            with tc.tile_pool(name="xstr", bufs=1) as xsp:
                xbs = []
                for kc in range(NKT):
                    xb = xsp.tile([P, S_SHARD], bf16, tag=f"xb{kc}")
                    nc.sync.dma_start(xb[:], t_xT[kc * P:(kc + 1) * P, :])
                    xbs.append(xb)
                for t in range(NST):
                    hps = psB.tile([P, P], f32, tag="acc")
                    for kc in range(NKT):
                        nc.tensor.matmul(hps[:], lhsT=xbs[kc][:, t * P:(t + 1) * P],
                                         rhs=W1_bf[kc][:],
                                         start=(kc == 0), stop=(kc == NKT - 1))
                    hp = sm.tile([P, P], bf16, tag="hp")
                    nc.vector.tensor_scalar(out=hp[:], in0=hps[:],
                                            scalar1=dinvs_sb[:, t:t + 1], scalar2=None,
                                            op0=ALU.mult)
                    nc.sync.dma_start(h_in[t * P:(t + 1) * P, :], hp[:])

            # ---- allgather ----
            nc.gpsimd.collective_compute(
                "AllGather", ALU.bypass,
                replica_groups=[list(range(NCORES))],
                ins=[h_in.opt()],
                outs=[h_full.opt()],
            )

            # phase-2 pools open after the x-streaming pool is released
            es = contextlib.ExitStack()
            gp = es.enter_context(tc.tile_pool(name="gat", bufs=1))
            ohp = es.enter_context(tc.tile_pool(name="ohp", bufs=1))

            # ---- phase 2: edge aggregation ----
            # gather ops per half: list of (chunk_base_slot, nchunks)
            def half_ops(L):
                ops = []
                base = 0
                while base < L:
                    n = min(OPCHUNKS * P, L - base)
                    ops.append((base, n))
                    base += n
                return ops

            opsA = half_ops(half_len[0])
            opsB = half_ops(half_len[1])
            gtiles = {0: {}, 1: {}}
            idx_sb = {0: idxA_sb, 1: idxB_sb}
            tabs = {0: h_full[0:HALF, :], 1: h_full[HALF:NT, :]}

            gq_counter = [0]

            def issue_gather(hh, opi, base, nsl):
                g = gp.tile([P, OPCHUNKS * P], bf16, tag=f"g{hh}", bufs=GBUFS)
                nc.gpsimd.dma_gather(
                    g[:, :nsl].rearrange("p (c f) -> p c f", f=P),
                    tabs[hh],
                    idx_sb[hh][:, base // 16: (base + nsl) // 16],
                    nsl, nsl, P,
                    queue_num=gq_counter[0] % NQUEUES,
                )
                gq_counter[0] += 1
                gtiles[hh][opi] = g

            for opi, (base, nsl) in enumerate(opsA):
                issue_gather(0, opi, base, nsl)
            for opi, (base, nsl) in enumerate(opsB):
                issue_gather(1, opi, base, nsl)

            # batched one-hot builds: one DVE op per OHG chunks
            oh_tiles = {}

            def onehot_group(g0):
                n = min(OHG, NPARTS - g0)
                oh = ohp.tile([P, OHG * P], bf16, tag="oh", bufs=8)
                nc.vector.tensor_tensor(
                    out=oh[:, :n * P].rearrange("p (c f) -> p c f", f=P),
                    in0=iota8_sb[:, :n * P].rearrange("p (c f) -> p c f", f=P),
                    in1=dcol_sb[:, g0:g0 + n].to_broadcast([P, n, P]),
                    op=ALU.is_equal)
                oh_tiles[g0] = oh

            winmax_sb = cst.tile([P, NWIN], f32)
            TRW = 4  # windows per transpose/reduce batch
            built_oh = set()

            def get_oh(ci):
                g0 = (ci // OHG) * OHG
                if g0 not in built_oh:
                    onehot_group(g0)
                    built_oh.add(g0)
                return oh_tiles[g0]

            pi = 0  # global part cursor into parts_tab / dstcol columns
            for w in range(NWIN):
                if w % TRW == 0:
                    tr = psA.tile([P, TRW * P], bf16, tag="tp")
                agg = psB.tile([P, P], f32, tag="acc")
                wparts = []
                while pi < NPARTS and parts_tab[pi][0] == w:
                    wparts.append((pi,) + tuple(parts_tab[pi][1:]))
                    pi += 1
                if need_bias:
                    # bias chunk: agg += sqrt(deg)[col] * b1[f]
                    nc.tensor.matmul(agg[:], lhsT=sdegT_bf[:1, w * P:(w + 1) * P],
                                     rhs=b1row_bf[:1, :], start=True, stop=False)
                for j, (pidx, hh, lc, a, b) in enumerate(wparts):
                    opi, off = divmod(lc * P, OPCHUNKS * P)
                    g = gtiles[hh][opi]
                    oh = get_oh(pidx)
                    ohc = (pidx % OHG) * P
                    nc.tensor.matmul(agg[:],
                                     lhsT=oh[:, ohc:ohc + P],
                                     rhs=g[:, off:off + P],
                                     start=(j == 0 and not need_bias),
                                     stop=(j == len(wparts) - 1))
                # hx = relu(dinv * (agg + sqrt(deg)*b1)) = relu(dinv*agg + b1)
                hx = sm.tile([P, P], bf16, tag="hx")
                nc.scalar.activation(hx[:], agg[:], AF.Relu,
                                     scale=dinvp_sb[:, w:w + 1])
                if need_mask:
                    nc.vector.tensor_scalar(out=hx[:], in0=hx[:],
                                            scalar1=mask_sb[:, w:w + 1], scalar2=None,
                                            op0=ALU.add)
                nc.tensor.transpose(tr[:, (w % TRW) * P:(w % TRW + 1) * P],
                                    hx[:], ident_bf[:])
                if w % TRW == TRW - 1:
                    nc.vector.reduce_max(
                        out=winmax_sb[:, w - TRW + 1:w + 1],
                        in_=tr[:].rearrange("p (c f) -> p c f", f=P), axis=AX.X)

            # ---- pooling: per-graph max over its windows ----
            pooled_sb = cst.tile([P, GPC], f32)
            for g in range(GPC):
                nc.vector.reduce_max(out=pooled_sb[:, g:g + 1],
                                     in_=winmax_sb[:, g * WPG:(g + 1) * WPG], axis=AX.X)

            # ---- news = relu(x_root @ W0 + b0) ----
            nps = psB.tile([GPC, P], f32, tag="acc")
            for kc in range(NKT):
                rt = sm.tile([P, GPC], f32, tag="rt")
                nc.sync.dma_start(rt[:], t_roots[kc * P:(kc + 1) * P, :])
                w0t = sm.tile([P, P], f32, tag="w0t")
                nc.sync.dma_start(w0t[:], t_W0[kc * P:(kc + 1) * P, :])
                nc.tensor.matmul(nps[:], lhsT=rt[:], rhs=w0t[:],
                                 start=(kc == 0), stop=(kc == NKT - 1))
            news = sm.tile([GPC, P], f32, tag="news")
            nc.vector.tensor_add(news[:], nps[:], b0_sb[:GPC, :])
            nc.scalar.activation(news[:], news[:], AF.Relu)
            ntr = psA.tile([P, GPC], f32, tag="tp")
            nc.tensor.transpose(ntr[:], news[:], ident_f[:GPC, :GPC])
            newsT = sm.tile([P, GPC], f32, tag="newsT")
            nc.vector.tensor_copy(newsT[:], ntr[:])

            # ---- z = relu([pooled|news] @ Wl1 + bl1) ----
            wl1a = sm.tile([P, P], f32, tag="wl1a")
            nc.sync.dma_start(wl1a[:], t_Wl1[0:P, :])
            wl1b = sm.tile([P, P], f32, tag="wl1b")
            nc.sync.dma_start(wl1b[:], t_Wl1[P:2 * P, :])
            zps = psB.tile([GPC, P], f32, tag="acc")
            nc.tensor.matmul(zps[:], lhsT=pooled_sb[:], rhs=wl1a[:], start=True, stop=False)
            nc.tensor.matmul(zps[:], lhsT=newsT[:], rhs=wl1b[:], start=False, stop=True)
            z2 = sm.tile([GPC, P], f32, tag="z2")
            nc.vector.tensor_add(z2[:], zps[:], bl1_sb[:GPC, :])
            nc.scalar.activation(z2[:], z2[:], AF.Relu)
            ztr = psA.tile([P, GPC], f32, tag="tp")
            nc.tensor.transpose(ztr[:], z2[:], ident_f[:GPC, :GPC])
            z2T = sm.tile([P, GPC], f32, tag="z2T")
            nc.vector.tensor_copy(z2T[:], ztr[:])

            # ---- logits + log_softmax ----
            wl2 = sm.tile([P, NCLS], f32, tag="wl2")
            nc.sync.dma_start(wl2[:], t_Wl2[:])
            lps = psB.tile([GPC, NCLS], f32, tag="acc")
            nc.tensor.matmul(lps[:], lhsT=z2T[:], rhs=wl2[:], start=True, stop=True)
            lg = sm.tile([GPC, NCLS], f32, tag="lg")
            nc.vector.tensor_add(lg[:], lps[:], bl2_sb[:GPC, :])
            mx = sm.tile([GPC, 1], f32, tag="mx")
            nc.vector.reduce_max(out=mx[:], in_=lg[:], axis=AX.X)
            tt = sm.tile([GPC, NCLS], f32, tag="tt")
            nc.vector.tensor_scalar(out=tt[:], in0=lg[:], scalar1=mx[:],
                                    scalar2=None, op0=ALU.subtract)
            ee = sm.tile([GPC, NCLS], f32, tag="ee")
            nc.scalar.activation(ee[:], tt[:], AF.Exp)
            ss = sm.tile([GPC, 1], f32, tag="ss")
            nc.vector.reduce_sum(out=ss[:], in_=ee[:], axis=AX.X)
            ls = sm.tile([GPC, 1], f32, tag="ls")
            nc.scalar.activation(ls[:], ss[:], AF.Ln)
            yy = sm.tile([GPC, NCLS], f32, tag="yy")
            nc.vector.tensor_scalar(out=yy[:], in0=tt[:], scalar1=ls[:],
                                    scalar2=None, op0=ALU.subtract)
            nc.sync.dma_start(t_y[:], yy[:])
            es.close()

    nc.compile()
    return nc


# ---------------------------------------------------------------------------
# entry point
# ---------------------------------------------------------------------------
def kernel(**inputs) -> np.ndarray:
    global LAST_EXEC_NS
    _install_ntff_hook()
    from concourse import bass_utils
    from concourse.bass_interp import get_hw_module

    x = np.asarray(inputs["x"], dtype=np.float32)
    ei = np.asarray(inputs["edge_index"])
    batch = np.asarray(inputs["batch"])
    G = int(np.asarray(inputs["num_graphs"]))
    W1 = np.asarray(inputs["W1"], dtype=np.float32)
    b1 = np.asarray(inputs["b1"], dtype=np.float32)
    W0 = np.asarray(inputs["W0"], dtype=np.float32)
    b0 = np.asarray(inputs["b0"], dtype=np.float32)
    Wl1 = np.asarray(inputs["Wl1"], dtype=np.float32)
    bl1 = np.asarray(inputs["bl1"], dtype=np.float32)
    Wl2 = np.asarray(inputs["Wl2"], dtype=np.float32)
    bl2 = np.asarray(inputs["bl2"], dtype=np.float32)
    NCLS = Wl2.shape[1]

    s = build_schedule(x, ei, batch, G)
    need_mask = bool((b1 > 0).any())
    need_bias = bool((b1 != 0).any())
    nc = build_program(s, NCLS, need_mask, need_bias)

    rep = lambda v, n: np.ascontiguousarray(np.tile(v[None, :], (n, 1)).astype(np.float32))
    iota_row = rep(np.arange(P, dtype=np.float32), P)
    iota8 = np.ascontiguousarray(np.tile(iota_row, (1, 8)))
    ident = np.eye(P, dtype=np.float32)

    in_maps = []
    for c in range(NCORES):
        in_maps.append({
            "xT_core": np.ascontiguousarray(s["x_core"][c].T).astype(ml_dtypes.bfloat16),
            "W1": W1, "W0": W0, "Wl1": Wl1, "Wl2": Wl2,
            "b1_rep": rep(b1, P), "b0_rep": rep(b0, P),
            "bl1_rep": rep(bl1, P), "bl2_rep": rep(bl2, P),
            "deg_shard": s["deg_shard"][c], "deg_pad": s["deg_pad"][c],
            "deg_padT": np.ascontiguousarray(s["deg_padT"][c].reshape(1, -1)),
            "maskneg": s["maskneg"][c], "rootsT": s["rootsT"][c],
            "idxA": s["idxA"][c], "idxB": s["idxB"][c],
            "dstcol": s["dstcol"][c],
            "iota_row": iota_row, "iota8": iota8, "identity": ident,
        })

    nc.m = get_hw_module(nc.m)
    res = bass_utils.run_bass_kernel_spmd(
        nc, in_maps, core_ids=list(range(NCORES)), trace=TRACE)
    LAST_EXEC_NS = res.exec_time_ns

    out = np.zeros((G, NCLS), np.float32)
    for c in range(NCORES):
        out[s["out_map"][c]] = res.results[c]["y"]
    return out



# revision 31
# speedup vs baseline: 3.8256x; 1.3746x over previous
"""TRN2 Bass kernel for nn_GCNModel: GCNConv + global max pool + root MLP head.

Strategy (8 NeuronCores, SPMD):
  - Graphs are assigned to cores (16 each, snake order by size for balance).
    Each core owns its graphs' nodes (contiguous ranges of the sorted batch).
  - h' = (x @ W1) * rsqrt(deg) computed shard-local in bf16 (PE, fp32 PSUM),
    then AllGather'd into a replicated row table [NT, 128] bf16 in DRAM.
  - Edge aggregation per core: edges (+ self loops) bucketed by
    (dst 128-node window, src table half), gathered row-wise from the table
    via gpsimd dma_gather (int16 half-local indices), then scattered into
    per-window PSUM accumulators with one-hot matmuls (exact, fp32 accum).
  - hx = relu(dinv * agg + b1) masked with -1e30 on pad rows; per-window
    column max after a PE transpose; per-graph max over its windows.
  - Head: news = relu(x_root @ W0 + b0); z = [pooled|news] @ Wl1 -> relu ->
    @ Wl2 -> log_softmax. All fp32.

The chunk schedule is made SPMD-uniform by padding per-(window,half) chunk
counts to the max over cores; pad lanes use dst column 200 (one-hot miss) so
they contribute exactly zero.
"""
import sys
import types
import contextlib
import ctypes

import numpy as np
import ml_dtypes

NCORES = 8
P = 128
OPCHUNKS = 8           # chunks (of 128 slots) per dma_gather op (1024 descs fits the ring)
NQUEUES = 4            # SWDGE queues; gathers round-robin so transfers overlap
GBUFS = 8              # in-flight gather buffers per table half
TRACE = False          # set by test.py for profiling
LAST_EXEC_NS = None


# ---------------------------------------------------------------------------
# axon NTFF profile hook (the image's antenv lacks axon_hooks)
# ---------------------------------------------------------------------------
def _install_ntff_hook():
    if "antenv.axon_hooks" in sys.modules:
        return
    try:
        lib = ctypes.CDLL("/opt/axon/libaxon_pjrt.so")
        if not hasattr(lib, "axon_start_nrt_profile"):
            return
    except OSError:
        return
    lib.axon_start_nrt_profile.argtypes = [ctypes.POINTER(ctypes.c_int64), ctypes.c_size_t]
    lib.axon_start_nrt_profile.restype = ctypes.c_int64
    lib.axon_stop_nrt_profile.argtypes = [ctypes.c_char_p]
    lib.axon_stop_nrt_profile.restype = ctypes.c_int64

    @contextlib.contextmanager
    def _hook(output_dir, device_ids):
        import jax
        jax.devices()
        if device_ids:
            ids = (ctypes.c_int64 * len(device_ids))(*device_ids)
            rc = lib.axon_start_nrt_profile(ids, len(device_ids))
        else:
            rc = lib.axon_start_nrt_profile(None, 0)
        if rc != 0:
            raise RuntimeError(f"axon_start_nrt_profile rc={rc}")
        try:
            yield
        finally:
            n = lib.axon_stop_nrt_profile(str(output_dir).encode())
            print(f"ntff profile: {n} file(s) -> {output_dir}", file=sys.stderr)

    mod = types.ModuleType("antenv.axon_hooks")
    mod.get_axon_ntff_profile_hook = lambda: _hook
    mod.set_axon_ntff_profile_hook = lambda h: None
    sys.modules["antenv.axon_hooks"] = mod


# ---------------------------------------------------------------------------
# host-side schedule
# ---------------------------------------------------------------------------
def build_schedule(x, edge_index, batch, num_graphs):
    N = x.shape[0]
    G = int(num_graphs)
    src = np.asarray(edge_index[0], dtype=np.int64)
    dst = np.asarray(edge_index[1], dtype=np.int64)
    batch = np.asarray(batch, dtype=np.int64)

    sizes = np.bincount(batch, minlength=G)
    gstart = np.zeros(G + 1, np.int64)
    np.cumsum(sizes, out=gstart[1:])

    first_idx = np.full(G, np.iinfo(np.int32).max, np.int64)
    np.minimum.at(first_idx, batch, np.arange(N))
    first_idx = np.clip(first_idx, 0, N - 1)

    deg = (np.bincount(dst, minlength=N) + 1).astype(np.float32)

    assert G % NCORES == 0, "graph count must divide core count"
    GPC = G // NCORES
    order = np.argsort(-sizes, kind="stable")
    g2core = np.zeros(G, np.int64)
    g2slot = np.zeros(G, np.int64)
    for i, g in enumerate(order):
        rnd, pos = divmod(i, NCORES)
        core = pos if rnd % 2 == 0 else NCORES - 1 - pos
        g2core[g] = core
        g2slot[g] = rnd

    S_GRAPH = max(512, int(np.ceil(sizes.max() / P)) * P)
    WPG = S_GRAPH // P
    NWIN = GPC * WPG

    core_graphs = []
    for c in range(NCORES):
        gs = [g for g in range(G) if g2core[g] == c]
        gs.sort(key=lambda g: g2slot[g])
        core_graphs.append(gs)
    real_c = np.array([sizes[core_graphs[c]].sum() for c in range(NCORES)])
    # 2 allgather chunks (the two gather halves) of whole 128-row tiles
    S_SHARD = int(np.ceil(real_c.max() / (2 * P))) * 2 * P
    CH = S_SHARD // 2
    NT = S_SHARD * NCORES
    HALF = NT // 2
    assert HALF % P == 0 and HALF < 32768, f"half table {HALF} must fit int16"

    # table rows are allgather-chunk-major: chunk j holds local rows
    # [j*CH, (j+1)*CH) of every core, concatenated in core order; chunk 0
    # is exactly table half A, chunk 1 half B.
    shard_row = np.zeros(N, np.int64)
    local_pos = np.zeros(N, np.int64)
    pad_w = np.zeros(N, np.int64)
    pad_col = np.zeros(N, np.int64)
    node_core = g2core[batch]
    for c in range(NCORES):
        pos = 0
        for g in core_graphs[c]:
            n0, n1 = gstart[g], gstart[g + 1]
            cnt = n1 - n0
            ar = np.arange(cnt)
            p = pos + ar
            jj = p // CH
            local_pos[n0:n1] = p
            shard_row[n0:n1] = jj * (NCORES * CH) + c * CH + (p - jj * CH)
            slot = g2slot[g]
            pad_w[n0:n1] = slot * WPG + ar // P
            pad_col[n0:n1] = ar % P
            pos += cnt

    # edge + self-loop lists per core
    ecore = node_core[dst]
    t_rows, t_wins, t_cols = [], [], []
    for c in range(NCORES):
        m = ecore == c
        nm = node_core == c
        t_rows.append(np.concatenate([shard_row[src[m]], shard_row[nm]]))
        t_wins.append(np.concatenate([pad_w[dst[m]], pad_w[nm]]))
        t_cols.append(np.concatenate([pad_col[dst[m]], pad_col[nm]]))

    counts = np.zeros((NCORES, NWIN, 2), np.int64)
    for c in range(NCORES):
        h = (t_rows[c] >= HALF).astype(np.int64)
        np.add.at(counts, (c, t_wins[c], h), 1)
    # tight segments: one per (window, half), max over cores, no 128-rounding
    seg_len = counts.max(axis=0)
    seg_len[:, 0] = np.maximum(seg_len[:, 0], 1)  # >=1 part so PSUM is written
    seg_off = np.zeros((NWIN, 2), np.int64)
    seg_off[1:, 0] = np.cumsum(seg_len[:-1, 0])
    seg_off[1:, 1] = np.cumsum(seg_len[:-1, 1])
    half_real = [int(seg_len[:, hh].sum()) for hh in (0, 1)]
    half_len = [-(-r // P) * P for r in half_real]
    TOTC_h = [half_len[hh] // P for hh in (0, 1)]
    TOTC = TOTC_h[0] + TOTC_h[1]

    # parts: per window (in order), the chunk ranges its segments occupy.
    # Each part scatters a full 128-slot chunk with cols outside [a,b)
    # masked to 200 (one-hot miss), so straddling chunks need no partition
    # slicing on the PE.
    parts = []  # (w, hh, lc, a, b)
    for w in range(NWIN):
        for hh in (0, 1):
            s0 = int(seg_off[w, hh])
            s1 = s0 + int(seg_len[w, hh])
            for lc in range(s0 // P, (s1 + P - 1) // P):
                a = max(0, s0 - lc * P)
                b = min(P, s1 - lc * P)
                parts.append((w, hh, lc, a, b))
    NPARTS = len(parts)

    src16 = [[None, None] for _ in range(NCORES)]
    dstcol = np.full((NCORES, P, NPARTS), 200.0, np.float32)
    for c in range(NCORES):
        rows, wins, cols = t_rows[c], t_wins[c], t_cols[c]
        h = (rows >= HALF).astype(np.int64)
        colstrs = {}
        for hh in (0, 1):
            stream = np.zeros(half_len[hh], np.int64)
            colstr = np.full(half_len[hh], 200.0, np.float32)
            for w in range(NWIN):
                m = (wins == w) & (h == hh)
                k = int(m.sum())
                base = int(seg_off[w, hh])
                srt = np.argsort(rows[m], kind="stable")
                stream[base:base + k] = rows[m][srt] - hh * HALF
                colstr[base:base + k] = cols[m][srt]
            src16[c][hh] = stream.astype(np.int16)
            colstrs[hh] = colstr
        for pi, (w, hh, lc, a, b) in enumerate(parts):
            dstcol[c, a:b, pi] = colstrs[hh][lc * P + a: lc * P + b]

    def wrap16(v):
        m = v.reshape(-1, 16).T  # [16, S]
        return np.ascontiguousarray(np.tile(m, (8, 1)).astype(np.int16))

    idx_w = [np.stack([wrap16(src16[c][hh]) for c in range(NCORES)]) for hh in (0, 1)]

    F = x.shape[1]
    x_core = np.zeros((NCORES, S_SHARD, F), np.float32)
    deg_padT = np.ones((NCORES, NWIN, P), np.float32)
    deg_shard = np.ones((NCORES, P, S_SHARD // P), np.float32)
    deg_pad = np.ones((NCORES, P, NWIN), np.float32)
    maskneg = np.full((NCORES, P, NWIN), -1e30, np.float32)
    rootsT = np.zeros((NCORES, F, GPC), np.float32)
    xf = np.asarray(x, dtype=np.float32)
    for c in range(NCORES):
        pos = 0
        for g in core_graphs[c]:
            n0, n1 = gstart[g], gstart[g + 1]
            cnt = n1 - n0
            x_core[c, pos:pos + cnt] = xf[n0:n1]
            pos += cnt
        nm = node_core == c
        sr = local_pos[nm]
        d = deg[nm]
        deg_shard[c, sr % P, sr // P] = d
        deg_pad[c, pad_col[nm], pad_w[nm]] = d
        deg_padT[c, pad_w[nm], pad_col[nm]] = d
        maskneg[c, pad_col[nm], pad_w[nm]] = 0.0
        for j, g in enumerate(core_graphs[c]):
            rootsT[c, :, j] = xf[first_idx[g]]

    out_map = np.array([core_graphs[c] for c in range(NCORES)])

    return dict(
        G=G, GPC=GPC, F=F, S_GRAPH=S_GRAPH, WPG=WPG, NWIN=NWIN,
        S_SHARD=S_SHARD, NT=NT, HALF=HALF,
        seg_len=seg_len, seg_off=seg_off, parts=parts, NPARTS=NPARTS,
        half_len=half_len, TOTC=TOTC, TOTC_h=TOTC_h,
        idxA=idx_w[0], idxB=idx_w[1], dstcol=dstcol,
        x_core=x_core, deg_shard=deg_shard, deg_pad=deg_pad,
        deg_padT=deg_padT,
        maskneg=maskneg, rootsT=rootsT, out_map=out_map,
    )


# ---------------------------------------------------------------------------
# bass program
# ---------------------------------------------------------------------------
def build_program(s, NCLS, need_mask, need_bias):
    import concourse.bass as bass
    import concourse.bacc as bacc
    import concourse.tile as tile
    import concourse.mybir as mybir

    f32 = mybir.dt.float32
    bf16 = mybir.dt.bfloat16
    i16 = mybir.dt.int16
    AF = mybir.ActivationFunctionType
    ALU = mybir.AluOpType
    AX = mybir.AxisListType

    F, GPC, NWIN, WPG = s["F"], s["GPC"], s["NWIN"], s["WPG"]
    S_SHARD, NT, HALF, TOTC = s["S_SHARD"], s["NT"], s["HALF"], s["TOTC"]
    NKT = F // P                 # k chunks for the 768-dim contraction
    NST = S_SHARD // P           # shard tiles
    seg_len = s["seg_len"]
    seg_off = s["seg_off"]
    half_len = s["half_len"]
    TOTC_h = s["TOTC_h"]
    parts_tab = s["parts"]
    NPARTS = s["NPARTS"]

    nc = bacc.Bacc("TRN2", target_bir_lowering=False, debug=False,
                   enable_asserts=False, num_devices=NCORES,
                   num_swdge_queues=NQUEUES,
                   dynamic_dma_scratch_size=32768)

    OHG = 8  # chunks per batched one-hot build

    # inputs
    t_xT = nc.dram_tensor("xT_core", [F, S_SHARD], bf16, kind="ExternalInput")
    t_W1 = nc.dram_tensor("W1", [F, P], f32, kind="ExternalInput")
    t_W0 = nc.dram_tensor("W0", [F, P], f32, kind="ExternalInput")
    t_Wl1 = nc.dram_tensor("Wl1", [2 * P, P], f32, kind="ExternalInput")
    t_Wl2 = nc.dram_tensor("Wl2", [P, NCLS], f32, kind="ExternalInput")
    t_b1 = nc.dram_tensor("b1_rep", [P, P], f32, kind="ExternalInput")
    t_b0 = nc.dram_tensor("b0_rep", [P, P], f32, kind="ExternalInput")
    t_bl1 = nc.dram_tensor("bl1_rep", [P, P], f32, kind="ExternalInput")
    t_bl2 = nc.dram_tensor("bl2_rep", [P, NCLS], f32, kind="ExternalInput")
    t_degs = nc.dram_tensor("deg_shard", [P, NST], f32, kind="ExternalInput")
    t_degp = nc.dram_tensor("deg_pad", [P, NWIN], f32, kind="ExternalInput")
    t_degpT = nc.dram_tensor("deg_padT", [1, NWIN * P], f32, kind="ExternalInput")
    t_mask = nc.dram_tensor("maskneg", [P, NWIN], f32, kind="ExternalInput")
    t_roots = nc.dram_tensor("rootsT", [F, GPC], f32, kind="ExternalInput")
    t_idxA = nc.dram_tensor("idxA", [P, half_len[0] // 16], i16, kind="ExternalInput")
    t_idxB = nc.dram_tensor("idxB", [P, half_len[1] // 16], i16, kind="ExternalInput")
    t_dcol = nc.dram_tensor("dstcol", [P, NPARTS], f32, kind="ExternalInput")
    t_iota = nc.dram_tensor("iota_row", [P, P], f32, kind="ExternalInput")
    t_iota8 = nc.dram_tensor("iota8", [P, OHG * P], f32, kind="ExternalInput")
    t_ident = nc.dram_tensor("identity", [P, P], f32, kind="ExternalInput")
    t_y = nc.dram_tensor("y", [GPC, NCLS], f32, kind="ExternalOutput")

    with tile.TileContext(nc) as tc:
        with tc.tile_pool(name="const", bufs=1) as cst, \
             tc.tile_pool(name="small", bufs=3) as sm, \
             tc.tile_pool(name="psA", bufs=2, space="PSUM") as psA, \
             tc.tile_pool(name="psB", bufs=2, space="PSUM") as psB, \
             tc.tile_pool(name="dram", bufs=1, space="DRAM") as dram:

            # ---- constants in SBUF ----
            ident_f = cst.tile([P, P], f32)
            nc.sync.dma_start(ident_f[:], t_ident[:])
            ident_bf = cst.tile([P, P], bf16)
            nc.vector.tensor_copy(ident_bf[:], ident_f[:])
            iota_sb = cst.tile([P, P], f32)
            nc.sync.dma_start(iota_sb[:], t_iota[:])
            iota8_sb = cst.tile([P, OHG * P], f32)
            nc.sync.dma_start(iota8_sb[:], t_iota8[:])
            b1_sb = cst.tile([P, P], f32)
            nc.sync.dma_start(b1_sb[:], t_b1[:])
            b0_sb = cst.tile([P, P], f32)
            nc.sync.dma_start(b0_sb[:], t_b0[:])
            bl1_sb = cst.tile([P, P], f32)
            nc.sync.dma_start(bl1_sb[:], t_bl1[:])
            bl2_sb = cst.tile([P, NCLS], f32)
            nc.sync.dma_start(bl2_sb[:], t_bl2[:])
            dcol_sb = cst.tile([P, NPARTS], f32)
            nc.sync.dma_start(dcol_sb[:], t_dcol[:])
            idxA_sb = cst.tile([P, half_len[0] // 16], i16)
            nc.sync.dma_start(idxA_sb[:], t_idxA[:])
            idxB_sb = cst.tile([P, half_len[1] // 16], i16)
            nc.sync.dma_start(idxB_sb[:], t_idxB[:])

            # dinv arrays: 1/sqrt(deg) = sqrt(1/deg)
            degs_sb = cst.tile([P, NST], f32)
            nc.sync.dma_start(degs_sb[:], t_degs[:])
            dinvs_sb = cst.tile([P, NST], f32)
            nc.vector.reciprocal(dinvs_sb[:], degs_sb[:])
            nc.scalar.activation(dinvs_sb[:], dinvs_sb[:], AF.Sqrt)
            degp_sb = cst.tile([P, NWIN], f32)
            nc.sync.dma_start(degp_sb[:], t_degp[:])
            dinvp_sb = cst.tile([P, NWIN], f32)
            nc.vector.reciprocal(dinvp_sb[:], degp_sb[:])
            nc.scalar.activation(dinvp_sb[:], dinvp_sb[:], AF.Sqrt)
            mask_sb = cst.tile([P, NWIN], f32)
            nc.sync.dma_start(mask_sb[:], t_mask[:])
            # sqrt(deg) per (window, col) transposed + b1 row, both bf16, for
            # the K=1 bias matmul injecting b1*sqrt(deg) into each window PSUM
            degpT_f = cst.tile([1, NWIN * P], f32)
            nc.sync.dma_start(degpT_f[:], t_degpT[:])
            nc.scalar.activation(degpT_f[:], degpT_f[:], AF.Sqrt)
            sdegT_bf = cst.tile([1, NWIN * P], bf16)
            nc.vector.tensor_copy(sdegT_bf[:], degpT_f[:])
            b1row_bf = cst.tile([1, P], bf16)
            nc.vector.tensor_copy(b1row_bf[:], b1_sb[:1, :])

            # W1 as bf16 k-chunk tiles
            W1_bf = []
            for kc in range(NKT):
                wt = cst.tile([P, P], f32, tag="w1f")
                nc.sync.dma_start(wt[:], t_W1[kc * P:(kc + 1) * P, :])
                wb = cst.tile([P, P], bf16, tag=f"w1b{kc}")
                nc.vector.tensor_copy(wb[:], wt[:])
                W1_bf.append(wb)

            # ---- phase 1: h' shard (x pre-transposed on host, bf16) ----
            # two column groups; each group's x loads, matmuls, and its
            # AllGather (one per table half) pipeline so the collective
            # overlaps compute and the half-A gathers.
            h_in = dram.tile([S_SHARD, P], bf16)
            CH = S_SHARD // 2
            CHT = CH // P  # tiles per allgather chunk
            h_halves = [dram.tile([NCORES * CH, P], bf16, addr_space="Shared",
                                  tag=f"hfull{j}", name=f"h_half{j}")
                        for j in range(2)]
            with tc.tile_pool(name="xstr", bufs=1) as xsp:
                xbs = []
                for kc in range(NKT):
                    xb = xsp.tile([P, S_SHARD], bf16, tag=f"xb{kc}")
                    xbs.append(xb)
                for jj in range(2):
                    for kc in range(NKT):
                        nc.sync.dma_start(xbs[kc][:, jj * CH:(jj + 1) * CH],
                                          t_xT[kc * P:(kc + 1) * P, jj * CH:(jj + 1) * CH])
                for jj in range(2):
                    for tt in range(CHT):
                        t = jj * CHT + tt
                        hps = psB.tile([P, P], f32, tag="acc")
                        for kc in range(NKT):
                            nc.tensor.matmul(hps[:], lhsT=xbs[kc][:, t * P:(t + 1) * P],
                                             rhs=W1_bf[kc][:],
                                             start=(kc == 0), stop=(kc == NKT - 1))
                        hp = sm.tile([P, P], bf16, tag="hp")
                        nc.vector.tensor_scalar(out=hp[:], in0=hps[:],
                                                scalar1=dinvs_sb[:, t:t + 1], scalar2=None,
                                                op0=ALU.mult)
                        nc.sync.dma_start(h_in[t * P:(t + 1) * P, :], hp[:])
                    # allgather half jj as soon as its rows are written
                    nc.gpsimd.collective_compute(
                        "AllGather", ALU.bypass,
                        replica_groups=[list(range(NCORES))],
                        ins=[h_in[jj * CH:(jj + 1) * CH, :].opt()],
                        outs=[h_halves[jj].opt()],
                    )

            # phase-2 pools open after the x-streaming pool is released
            es = contextlib.ExitStack()
            gp = es.enter_context(tc.tile_pool(name="gat", bufs=1))
            ohp = es.enter_context(tc.tile_pool(name="ohp", bufs=1))

            # ---- phase 2: edge aggregation ----
            # gather ops per half: list of (chunk_base_slot, nchunks)
            def half_ops(L):
                ops = []
                base = 0
                while base < L:
                    n = min(OPCHUNKS * P, L - base)
                    ops.append((base, n))
                    base += n
                return ops

            opsA = half_ops(half_len[0])
            opsB = half_ops(half_len[1])
            gtiles = {0: {}, 1: {}}
            idx_sb = {0: idxA_sb, 1: idxB_sb}
            tabs = {0: h_halves[0][:], 1: h_halves[1][:]}

            gq_counter = [0]

            def issue_gather(hh, opi, base, nsl):
                g = gp.tile([P, OPCHUNKS * P], bf16, tag=f"g{hh}", bufs=GBUFS)
                nc.gpsimd.dma_gather(
                    g[:, :nsl].rearrange("p (c f) -> p c f", f=P),
                    tabs[hh],
                    idx_sb[hh][:, base // 16: (base + nsl) // 16],
                    nsl, nsl, P,
                    queue_num=gq_counter[0] % NQUEUES,
                )
                gq_counter[0] += 1
                gtiles[hh][opi] = g

            # issue gathers in consumption order (windows interleave the two
            # halves, so strict A-then-B issue order head-of-line blocks the
            # buffer rings)
            allops = []
            for hh, ops in ((0, opsA), (1, opsB)):
                for opi, (base, nsl) in enumerate(ops):
                    fw = int(np.searchsorted(seg_off[:, hh], base, side="right")) - 1
                    allops.append((fw, hh, opi, base, nsl))
            allops.sort()
            for fw, hh, opi, base, nsl in allops:
                issue_gather(hh, opi, base, nsl)

            # batched one-hot builds: one DVE op per OHG chunks
            oh_tiles = {}

            def onehot_group(g0):
                n = min(OHG, NPARTS - g0)
                oh = ohp.tile([P, OHG * P], bf16, tag="oh", bufs=10)
                nc.vector.tensor_tensor(
                    out=oh[:, :n * P].rearrange("p (c f) -> p c f", f=P),
                    in0=iota8_sb[:, :n * P].rearrange("p (c f) -> p c f", f=P),
                    in1=dcol_sb[:, g0:g0 + n].to_broadcast([P, n, P]),
                    op=ALU.is_equal)
                oh_tiles[g0] = oh

            winmax_sb = cst.tile([P, NWIN], f32)
            TRW = 4  # windows per transpose/reduce batch
            built_oh = set()

            def get_oh(ci):
                g0 = (ci // OHG) * OHG
                if g0 not in built_oh:
                    onehot_group(g0)
                    built_oh.add(g0)
                return oh_tiles[g0]

            pi = 0  # global part cursor into parts_tab / dstcol columns
            for w in range(NWIN):
                if w % TRW == 0:
                    tr = psA.tile([P, TRW * P], bf16, tag="tp")
                agg = psB.tile([P, P], f32, tag="acc")
                wparts = []
                while pi < NPARTS and parts_tab[pi][0] == w:
                    wparts.append((pi,) + tuple(parts_tab[pi][1:]))
                    pi += 1
                if need_bias:
                    # bias chunk: agg += sqrt(deg)[col] * b1[f]
                    nc.tensor.matmul(agg[:], lhsT=sdegT_bf[:1, w * P:(w + 1) * P],
                                     rhs=b1row_bf[:1, :], start=True, stop=False)
                for j, (pidx, hh, lc, a, b) in enumerate(wparts):
                    opi, off = divmod(lc * P, OPCHUNKS * P)
                    g = gtiles[hh][opi]
                    oh = get_oh(pidx)
                    ohc = (pidx % OHG) * P
                    nc.tensor.matmul(agg[:],
                                     lhsT=oh[:, ohc:ohc + P],
                                     rhs=g[:, off:off + P],
                                     start=(j == 0 and not need_bias),
                                     stop=(j == len(wparts) - 1))
                # hx = relu(dinv * (agg + sqrt(deg)*b1)) = relu(dinv*agg + b1)
                hx = sm.tile([P, P], bf16, tag="hx")
                nc.scalar.activation(hx[:], agg[:], AF.Relu,
                                     scale=dinvp_sb[:, w:w + 1])
                if need_mask:
                    nc.vector.tensor_scalar(out=hx[:], in0=hx[:],
                                            scalar1=mask_sb[:, w:w + 1], scalar2=None,
                                            op0=ALU.add)
                nc.tensor.transpose(tr[:, (w % TRW) * P:(w % TRW + 1) * P],
                                    hx[:], ident_bf[:])
                if w % TRW == TRW - 1:
                    nc.vector.reduce_max(
                        out=winmax_sb[:, w - TRW + 1:w + 1],
                        in_=tr[:].rearrange("p (c f) -> p c f", f=P), axis=AX.X)

            # ---- pooling: per-graph max over its windows ----
            pooled_sb = cst.tile([P, GPC], f32)
            for g in range(GPC):
                nc.vector.reduce_max(out=pooled_sb[:, g:g + 1],
                                     in_=winmax_sb[:, g * WPG:(g + 1) * WPG], axis=AX.X)

            # ---- news = relu(x_root @ W0 + b0) ----
            nps = psB.tile([GPC, P], f32, tag="acc")
            for kc in range(NKT):
                rt = sm.tile([P, GPC], f32, tag="rt")
                nc.sync.dma_start(rt[:], t_roots[kc * P:(kc + 1) * P, :])
                w0t = sm.tile([P, P], f32, tag="w0t")
                nc.sync.dma_start(w0t[:], t_W0[kc * P:(kc + 1) * P, :])
                nc.tensor.matmul(nps[:], lhsT=rt[:], rhs=w0t[:],
                                 start=(kc == 0), stop=(kc == NKT - 1))
            news = sm.tile([GPC, P], f32, tag="news")
            nc.vector.tensor_add(news[:], nps[:], b0_sb[:GPC, :])
            nc.scalar.activation(news[:], news[:], AF.Relu)
            ntr = psA.tile([P, GPC], f32, tag="tp")
            nc.tensor.transpose(ntr[:], news[:], ident_f[:GPC, :GPC])
            newsT = sm.tile([P, GPC], f32, tag="newsT")
            nc.vector.tensor_copy(newsT[:], ntr[:])

            # ---- z = relu([pooled|news] @ Wl1 + bl1) ----
            wl1a = sm.tile([P, P], f32, tag="wl1a")
            nc.sync.dma_start(wl1a[:], t_Wl1[0:P, :])
            wl1b = sm.tile([P, P], f32, tag="wl1b")
            nc.sync.dma_start(wl1b[:], t_Wl1[P:2 * P, :])
            zps = psB.tile([GPC, P], f32, tag="acc")
            nc.tensor.matmul(zps[:], lhsT=pooled_sb[:], rhs=wl1a[:], start=True, stop=False)
            nc.tensor.matmul(zps[:], lhsT=newsT[:], rhs=wl1b[:], start=False, stop=True)
            z2 = sm.tile([GPC, P], f32, tag="z2")
            nc.vector.tensor_add(z2[:], zps[:], bl1_sb[:GPC, :])
            nc.scalar.activation(z2[:], z2[:], AF.Relu)
            ztr = psA.tile([P, GPC], f32, tag="tp")
            nc.tensor.transpose(ztr[:], z2[:], ident_f[:GPC, :GPC])
            z2T = sm.tile([P, GPC], f32, tag="z2T")
            nc.vector.tensor_copy(z2T[:], ztr[:])

            # ---- logits + log_softmax ----
            wl2 = sm.tile([P, NCLS], f32, tag="wl2")
            nc.sync.dma_start(wl2[:], t_Wl2[:])
            lps = psB.tile([GPC, NCLS], f32, tag="acc")
            nc.tensor.matmul(lps[:], lhsT=z2T[:], rhs=wl2[:], start=True, stop=True)
            lg = sm.tile([GPC, NCLS], f32, tag="lg")
            nc.vector.tensor_add(lg[:], lps[:], bl2_sb[:GPC, :])
            mx = sm.tile([GPC, 1], f32, tag="mx")
            nc.vector.reduce_max(out=mx[:], in_=lg[:], axis=AX.X)
            tt = sm.tile([GPC, NCLS], f32, tag="tt")
            nc.vector.tensor_scalar(out=tt[:], in0=lg[:], scalar1=mx[:],
                                    scalar2=None, op0=ALU.subtract)
            ee = sm.tile([GPC, NCLS], f32, tag="ee")
            nc.scalar.activation(ee[:], tt[:], AF.Exp)
            ss = sm.tile([GPC, 1], f32, tag="ss")
            nc.vector.reduce_sum(out=ss[:], in_=ee[:], axis=AX.X)
            ls = sm.tile([GPC, 1], f32, tag="ls")
            nc.scalar.activation(ls[:], ss[:], AF.Ln)
            yy = sm.tile([GPC, NCLS], f32, tag="yy")
            nc.vector.tensor_scalar(out=yy[:], in0=tt[:], scalar1=ls[:],
                                    scalar2=None, op0=ALU.subtract)
            nc.sync.dma_start(t_y[:], yy[:])
            es.close()

    nc.compile()
    return nc


# ---------------------------------------------------------------------------
# entry point
# ---------------------------------------------------------------------------
def kernel(**inputs) -> np.ndarray:
    global LAST_EXEC_NS
    _install_ntff_hook()
    from concourse import bass_utils
    from concourse.bass_interp import get_hw_module

    x = np.asarray(inputs["x"], dtype=np.float32)
    ei = np.asarray(inputs["edge_index"])
    batch = np.asarray(inputs["batch"])
    G = int(np.asarray(inputs["num_graphs"]))
    W1 = np.asarray(inputs["W1"], dtype=np.float32)
    b1 = np.asarray(inputs["b1"], dtype=np.float32)
    W0 = np.asarray(inputs["W0"], dtype=np.float32)
    b0 = np.asarray(inputs["b0"], dtype=np.float32)
    Wl1 = np.asarray(inputs["Wl1"], dtype=np.float32)
    bl1 = np.asarray(inputs["bl1"], dtype=np.float32)
    Wl2 = np.asarray(inputs["Wl2"], dtype=np.float32)
    bl2 = np.asarray(inputs["bl2"], dtype=np.float32)
    NCLS = Wl2.shape[1]

    s = build_schedule(x, ei, batch, G)
    need_mask = bool((b1 > 0).any())
    need_bias = bool((b1 != 0).any())
    nc = build_program(s, NCLS, need_mask, need_bias)

    rep = lambda v, n: np.ascontiguousarray(np.tile(v[None, :], (n, 1)).astype(np.float32))
    iota_row = rep(np.arange(P, dtype=np.float32), P)
    iota8 = np.ascontiguousarray(np.tile(iota_row, (1, 8)))
    ident = np.eye(P, dtype=np.float32)

    in_maps = []
    for c in range(NCORES):
        in_maps.append({
            "xT_core": np.ascontiguousarray(s["x_core"][c].T).astype(ml_dtypes.bfloat16),
            "W1": W1, "W0": W0, "Wl1": Wl1, "Wl2": Wl2,
            "b1_rep": rep(b1, P), "b0_rep": rep(b0, P),
            "bl1_rep": rep(bl1, P), "bl2_rep": rep(bl2, P),
            "deg_shard": s["deg_shard"][c], "deg_pad": s["deg_pad"][c],
            "deg_padT": np.ascontiguousarray(s["deg_padT"][c].reshape(1, -1)),
            "maskneg": s["maskneg"][c], "rootsT": s["rootsT"][c],
            "idxA": s["idxA"][c], "idxB": s["idxB"][c],
            "dstcol": s["dstcol"][c],
            "iota_row": iota_row, "iota8": iota8, "identity": ident,
        })

    nc.m = get_hw_module(nc.m)
    res = bass_utils.run_bass_kernel_spmd(
        nc, in_maps, core_ids=list(range(NCORES)), trace=TRACE)
    LAST_EXEC_NS = res.exec_time_ns

    out = np.zeros((G, NCLS), np.float32)
    for c in range(NCORES):
        out[s["out_map"][c]] = res.results[c]["y"]
    return out



# revision 34
# speedup vs baseline: 3.8317x; 1.0016x over previous
"""TRN2 Bass kernel for nn_GCNModel: GCNConv + global max pool + root MLP head.

Strategy (8 NeuronCores, SPMD):
  - Graphs are assigned to cores (16 each, snake order by size for balance).
    Each core owns its graphs' nodes (contiguous ranges of the sorted batch).
  - h' = (x @ W1) * rsqrt(deg) computed shard-local in bf16 (PE, fp32 PSUM),
    then AllGather'd into a replicated row table [NT, 128] bf16 in DRAM.
  - Edge aggregation per core: edges (+ self loops) bucketed by
    (dst 128-node window, src table half), gathered row-wise from the table
    via gpsimd dma_gather (int16 half-local indices), then scattered into
    per-window PSUM accumulators with one-hot matmuls (exact, fp32 accum).
  - hx = relu(dinv * agg + b1) masked with -1e30 on pad rows; per-window
    column max after a PE transpose; per-graph max over its windows.
  - Head: news = relu(x_root @ W0 + b0); z = [pooled|news] @ Wl1 -> relu ->
    @ Wl2 -> log_softmax. All fp32.

The chunk schedule is made SPMD-uniform by padding per-(window,half) chunk
counts to the max over cores; pad lanes use dst column 200 (one-hot miss) so
they contribute exactly zero.
"""
import sys
import types
import contextlib
import ctypes

import numpy as np
import ml_dtypes

NCORES = 8
P = 128
OPCHUNKS = 8           # chunks (of 128 slots) per dma_gather op (1024 descs fits the ring)
NQUEUES = 4            # SWDGE queues; gathers round-robin so transfers overlap
GBUFS = 10             # in-flight gather buffers per table half
TRACE = False          # set by test.py for profiling
LAST_EXEC_NS = None


# ---------------------------------------------------------------------------
# axon NTFF profile hook (the image's antenv lacks axon_hooks)
# ---------------------------------------------------------------------------
def _install_ntff_hook():
    if "antenv.axon_hooks" in sys.modules:
        return
    try:
        lib = ctypes.CDLL("/opt/axon/libaxon_pjrt.so")
        if not hasattr(lib, "axon_start_nrt_profile"):
            return
    except OSError:
        return
    lib.axon_start_nrt_profile.argtypes = [ctypes.POINTER(ctypes.c_int64), ctypes.c_size_t]
    lib.axon_start_nrt_profile.restype = ctypes.c_int64
    lib.axon_stop_nrt_profile.argtypes = [ctypes.c_char_p]
    lib.axon_stop_nrt_profile.restype = ctypes.c_int64

    @contextlib.contextmanager
    def _hook(output_dir, device_ids):
        import jax
        jax.devices()
        if device_ids:
            ids = (ctypes.c_int64 * len(device_ids))(*device_ids)
            rc = lib.axon_start_nrt_profile(ids, len(device_ids))
        else:
            rc = lib.axon_start_nrt_profile(None, 0)
        if rc != 0:
            raise RuntimeError(f"axon_start_nrt_profile rc={rc}")
        try:
            yield
        finally:
            n = lib.axon_stop_nrt_profile(str(output_dir).encode())
            print(f"ntff profile: {n} file(s) -> {output_dir}", file=sys.stderr)

    mod = types.ModuleType("antenv.axon_hooks")
    mod.get_axon_ntff_profile_hook = lambda: _hook
    mod.set_axon_ntff_profile_hook = lambda h: None
    sys.modules["antenv.axon_hooks"] = mod


# ---------------------------------------------------------------------------
# host-side schedule
# ---------------------------------------------------------------------------
def build_schedule(x, edge_index, batch, num_graphs):
    N = x.shape[0]
    G = int(num_graphs)
    src = np.asarray(edge_index[0], dtype=np.int64)
    dst = np.asarray(edge_index[1], dtype=np.int64)
    batch = np.asarray(batch, dtype=np.int64)

    sizes = np.bincount(batch, minlength=G)
    gstart = np.zeros(G + 1, np.int64)
    np.cumsum(sizes, out=gstart[1:])

    first_idx = np.full(G, np.iinfo(np.int32).max, np.int64)
    np.minimum.at(first_idx, batch, np.arange(N))
    first_idx = np.clip(first_idx, 0, N - 1)

    deg = (np.bincount(dst, minlength=N) + 1).astype(np.float32)

    assert G % NCORES == 0, "graph count must divide core count"
    GPC = G // NCORES
    order = np.argsort(-sizes, kind="stable")
    g2core = np.zeros(G, np.int64)
    g2slot = np.zeros(G, np.int64)
    for i, g in enumerate(order):
        rnd, pos = divmod(i, NCORES)
        core = pos if rnd % 2 == 0 else NCORES - 1 - pos
        g2core[g] = core
        g2slot[g] = rnd

    S_GRAPH = max(512, int(np.ceil(sizes.max() / P)) * P)
    WPG = S_GRAPH // P
    NWIN = GPC * WPG

    core_graphs = []
    for c in range(NCORES):
        gs = [g for g in range(G) if g2core[g] == c]
        gs.sort(key=lambda g: g2slot[g])
        core_graphs.append(gs)
    real_c = np.array([sizes[core_graphs[c]].sum() for c in range(NCORES)])
    # 2 allgather chunks (the two gather halves) of whole 128-row tiles
    S_SHARD = int(np.ceil(real_c.max() / (2 * P))) * 2 * P
    CH = S_SHARD // 2
    NT = S_SHARD * NCORES
    HALF = NT // 2
    assert HALF % P == 0 and HALF < 32768, f"half table {HALF} must fit int16"

    # table rows are allgather-chunk-major: chunk j holds local rows
    # [j*CH, (j+1)*CH) of every core, concatenated in core order; chunk 0
    # is exactly table half A, chunk 1 half B.
    shard_row = np.zeros(N, np.int64)
    local_pos = np.zeros(N, np.int64)
    pad_w = np.zeros(N, np.int64)
    pad_col = np.zeros(N, np.int64)
    node_core = g2core[batch]
    for c in range(NCORES):
        pos = 0
        for g in core_graphs[c]:
            n0, n1 = gstart[g], gstart[g + 1]
            cnt = n1 - n0
            ar = np.arange(cnt)
            p = pos + ar
            jj = p // CH
            local_pos[n0:n1] = p
            shard_row[n0:n1] = jj * (NCORES * CH) + c * CH + (p - jj * CH)
            slot = g2slot[g]
            pad_w[n0:n1] = slot * WPG + ar // P
            pad_col[n0:n1] = ar % P
            pos += cnt

    # edge + self-loop lists per core
    ecore = node_core[dst]
    t_rows, t_wins, t_cols = [], [], []
    for c in range(NCORES):
        m = ecore == c
        nm = node_core == c
        t_rows.append(np.concatenate([shard_row[src[m]], shard_row[nm]]))
        t_wins.append(np.concatenate([pad_w[dst[m]], pad_w[nm]]))
        t_cols.append(np.concatenate([pad_col[dst[m]], pad_col[nm]]))

    counts = np.zeros((NCORES, NWIN, 2), np.int64)
    for c in range(NCORES):
        h = (t_rows[c] >= HALF).astype(np.int64)
        np.add.at(counts, (c, t_wins[c], h), 1)
    # tight segments: one per (window, half), max over cores, no 128-rounding
    seg_len = counts.max(axis=0)
    seg_len[:, 0] = np.maximum(seg_len[:, 0], 1)  # >=1 part so PSUM is written
    seg_off = np.zeros((NWIN, 2), np.int64)
    seg_off[1:, 0] = np.cumsum(seg_len[:-1, 0])
    seg_off[1:, 1] = np.cumsum(seg_len[:-1, 1])
    half_real = [int(seg_len[:, hh].sum()) for hh in (0, 1)]
    half_len = [-(-r // P) * P for r in half_real]
    TOTC_h = [half_len[hh] // P for hh in (0, 1)]
    TOTC = TOTC_h[0] + TOTC_h[1]

    # parts: per window (in order), the chunk ranges its segments occupy.
    # Each part scatters a full 128-slot chunk with cols outside [a,b)
    # masked to 200 (one-hot miss), so straddling chunks need no partition
    # slicing on the PE.
    parts = []  # (w, hh, lc, a, b)
    for w in range(NWIN):
        for hh in (0, 1):
            s0 = int(seg_off[w, hh])
            s1 = s0 + int(seg_len[w, hh])
            for lc in range(s0 // P, (s1 + P - 1) // P):
                a = max(0, s0 - lc * P)
                b = min(P, s1 - lc * P)
                parts.append((w, hh, lc, a, b))
    NPARTS = len(parts)

    src16 = [[None, None] for _ in range(NCORES)]
    dstcol = np.full((NCORES, P, NPARTS), 200.0, np.float32)
    for c in range(NCORES):
        rows, wins, cols = t_rows[c], t_wins[c], t_cols[c]
        h = (rows >= HALF).astype(np.int64)
        colstrs = {}
        for hh in (0, 1):
            stream = np.zeros(half_len[hh], np.int64)
            colstr = np.full(half_len[hh], 200.0, np.float32)
            for w in range(NWIN):
                m = (wins == w) & (h == hh)
                k = int(m.sum())
                base = int(seg_off[w, hh])
                srt = np.argsort(rows[m], kind="stable")
                stream[base:base + k] = rows[m][srt] - hh * HALF
                colstr[base:base + k] = cols[m][srt]
            src16[c][hh] = stream.astype(np.int16)
            colstrs[hh] = colstr
        for pi, (w, hh, lc, a, b) in enumerate(parts):
            dstcol[c, a:b, pi] = colstrs[hh][lc * P + a: lc * P + b]

    def wrap16(v):
        m = v.reshape(-1, 16).T  # [16, S]
        return np.ascontiguousarray(np.tile(m, (8, 1)).astype(np.int16))

    idx_w = [np.stack([wrap16(src16[c][hh]) for c in range(NCORES)]) for hh in (0, 1)]

    F = x.shape[1]
    x_core = np.zeros((NCORES, S_SHARD, F), np.float32)
    deg_padT = np.ones((NCORES, NWIN, P), np.float32)
    deg_shard = np.ones((NCORES, P, S_SHARD // P), np.float32)
    deg_pad = np.ones((NCORES, P, NWIN), np.float32)
    maskneg = np.full((NCORES, P, NWIN), -1e30, np.float32)
    rootsT = np.zeros((NCORES, F, GPC), np.float32)
    xf = np.asarray(x, dtype=np.float32)
    for c in range(NCORES):
        pos = 0
        for g in core_graphs[c]:
            n0, n1 = gstart[g], gstart[g + 1]
            cnt = n1 - n0
            x_core[c, pos:pos + cnt] = xf[n0:n1]
            pos += cnt
        nm = node_core == c
        sr = local_pos[nm]
        d = deg[nm]
        deg_shard[c, sr % P, sr // P] = d
        deg_pad[c, pad_col[nm], pad_w[nm]] = d
        deg_padT[c, pad_w[nm], pad_col[nm]] = d
        maskneg[c, pad_col[nm], pad_w[nm]] = 0.0
        for j, g in enumerate(core_graphs[c]):
            rootsT[c, :, j] = xf[first_idx[g]]

    out_map = np.array([core_graphs[c] for c in range(NCORES)])

    return dict(
        G=G, GPC=GPC, F=F, S_GRAPH=S_GRAPH, WPG=WPG, NWIN=NWIN,
        S_SHARD=S_SHARD, NT=NT, HALF=HALF,
        seg_len=seg_len, seg_off=seg_off, parts=parts, NPARTS=NPARTS,
        half_len=half_len, TOTC=TOTC, TOTC_h=TOTC_h,
        idxA=idx_w[0], idxB=idx_w[1], dstcol=dstcol,
        x_core=x_core, deg_shard=deg_shard, deg_pad=deg_pad,
        deg_padT=deg_padT,
        maskneg=maskneg, rootsT=rootsT, out_map=out_map,
    )


# ---------------------------------------------------------------------------
# bass program
# ---------------------------------------------------------------------------
def build_program(s, NCLS, need_mask, need_bias):
    import concourse.bass as bass
    import concourse.bacc as bacc
    import concourse.tile as tile
    import concourse.mybir as mybir

    f32 = mybir.dt.float32
    bf16 = mybir.dt.bfloat16
    i16 = mybir.dt.int16
    AF = mybir.ActivationFunctionType
    ALU = mybir.AluOpType
    AX = mybir.AxisListType

    F, GPC, NWIN, WPG = s["F"], s["GPC"], s["NWIN"], s["WPG"]
    S_SHARD, NT, HALF, TOTC = s["S_SHARD"], s["NT"], s["HALF"], s["TOTC"]
    NKT = F // P                 # k chunks for the 768-dim contraction
    NST = S_SHARD // P           # shard tiles
    seg_len = s["seg_len"]
    seg_off = s["seg_off"]
    half_len = s["half_len"]
    TOTC_h = s["TOTC_h"]
    parts_tab = s["parts"]
    NPARTS = s["NPARTS"]

    nc = bacc.Bacc("TRN2", target_bir_lowering=False, debug=False,
                   enable_asserts=False, num_devices=NCORES,
                   num_swdge_queues=NQUEUES,
                   dynamic_dma_scratch_size=49152)

    OHG = 8  # chunks per batched one-hot build

    # inputs
    t_xT = nc.dram_tensor("xT_core", [F, S_SHARD], bf16, kind="ExternalInput")
    t_W1 = nc.dram_tensor("W1", [F, P], f32, kind="ExternalInput")
    t_W0 = nc.dram_tensor("W0", [F, P], f32, kind="ExternalInput")
    t_Wl1 = nc.dram_tensor("Wl1", [2 * P, P], f32, kind="ExternalInput")
    t_Wl2 = nc.dram_tensor("Wl2", [P, NCLS], f32, kind="ExternalInput")
    t_b1 = nc.dram_tensor("b1_rep", [P, P], f32, kind="ExternalInput")
    t_b0 = nc.dram_tensor("b0_rep", [P, P], f32, kind="ExternalInput")
    t_bl1 = nc.dram_tensor("bl1_rep", [P, P], f32, kind="ExternalInput")
    t_bl2 = nc.dram_tensor("bl2_rep", [P, NCLS], f32, kind="ExternalInput")
    t_degs = nc.dram_tensor("deg_shard", [P, NST], f32, kind="ExternalInput")
    t_degp = nc.dram_tensor("deg_pad", [P, NWIN], f32, kind="ExternalInput")
    t_degpT = nc.dram_tensor("deg_padT", [1, NWIN * P], f32, kind="ExternalInput")
    t_mask = nc.dram_tensor("maskneg", [P, NWIN], f32, kind="ExternalInput")
    t_roots = nc.dram_tensor("rootsT", [F, GPC], f32, kind="ExternalInput")
    t_idxA = nc.dram_tensor("idxA", [P, half_len[0] // 16], i16, kind="ExternalInput")
    t_idxB = nc.dram_tensor("idxB", [P, half_len[1] // 16], i16, kind="ExternalInput")
    t_dcol = nc.dram_tensor("dstcol", [P, NPARTS], f32, kind="ExternalInput")
    t_iota = nc.dram_tensor("iota_row", [P, P], f32, kind="ExternalInput")
    t_iota8 = nc.dram_tensor("iota8", [P, OHG * P], f32, kind="ExternalInput")
    t_ident = nc.dram_tensor("identity", [P, P], f32, kind="ExternalInput")
    t_y = nc.dram_tensor("y", [GPC, NCLS], f32, kind="ExternalOutput")

    with tile.TileContext(nc) as tc:
        with tc.tile_pool(name="const", bufs=1) as cst, \
             tc.tile_pool(name="small", bufs=3) as sm, \
             tc.tile_pool(name="psA", bufs=2, space="PSUM") as psA, \
             tc.tile_pool(name="psB", bufs=2, space="PSUM") as psB, \
             tc.tile_pool(name="dram", bufs=1, space="DRAM") as dram:

            # ---- constants in SBUF ----
            ident_f = cst.tile([P, P], f32)
            nc.sync.dma_start(ident_f[:], t_ident[:])
            ident_bf = cst.tile([P, P], bf16)
            nc.vector.tensor_copy(ident_bf[:], ident_f[:])
            iota_sb = cst.tile([P, P], f32)
            nc.sync.dma_start(iota_sb[:], t_iota[:])
            iota8_sb = cst.tile([P, OHG * P], f32)
            nc.sync.dma_start(iota8_sb[:], t_iota8[:])
            b1_sb = cst.tile([P, P], f32)
            nc.sync.dma_start(b1_sb[:], t_b1[:])
            b0_sb = cst.tile([P, P], f32)
            nc.sync.dma_start(b0_sb[:], t_b0[:])
            bl1_sb = cst.tile([P, P], f32)
            nc.sync.dma_start(bl1_sb[:], t_bl1[:])
            bl2_sb = cst.tile([P, NCLS], f32)
            nc.sync.dma_start(bl2_sb[:], t_bl2[:])
            dcol_sb = cst.tile([P, NPARTS], f32)
            nc.gpsimd.dma_start(dcol_sb[:], t_dcol[:])
            idxA_sb = cst.tile([P, half_len[0] // 16], i16)
            nc.gpsimd.dma_start(idxA_sb[:], t_idxA[:])
            idxB_sb = cst.tile([P, half_len[1] // 16], i16)
            nc.gpsimd.dma_start(idxB_sb[:], t_idxB[:])

            # dinv arrays: 1/sqrt(deg) = sqrt(1/deg)
            degs_sb = cst.tile([P, NST], f32)
            nc.sync.dma_start(degs_sb[:], t_degs[:])
            dinvs_sb = cst.tile([P, NST], f32)
            nc.vector.reciprocal(dinvs_sb[:], degs_sb[:])
            nc.scalar.activation(dinvs_sb[:], dinvs_sb[:], AF.Sqrt)
            degp_sb = cst.tile([P, NWIN], f32)
            nc.sync.dma_start(degp_sb[:], t_degp[:])
            dinvp_sb = cst.tile([P, NWIN], f32)
            nc.vector.reciprocal(dinvp_sb[:], degp_sb[:])
            nc.scalar.activation(dinvp_sb[:], dinvp_sb[:], AF.Sqrt)
            mask_sb = cst.tile([P, NWIN], f32)
            nc.sync.dma_start(mask_sb[:], t_mask[:])
            # sqrt(deg) per (window, col) transposed + b1 row, both bf16, for
            # the K=1 bias matmul injecting b1*sqrt(deg) into each window PSUM
            degpT_f = cst.tile([1, NWIN * P], f32)
            nc.sync.dma_start(degpT_f[:], t_degpT[:])
            nc.scalar.activation(degpT_f[:], degpT_f[:], AF.Sqrt)
            sdegT_bf = cst.tile([1, NWIN * P], bf16)
            nc.vector.tensor_copy(sdegT_bf[:], degpT_f[:])
            b1row_bf = cst.tile([1, P], bf16)
            nc.vector.tensor_copy(b1row_bf[:], b1_sb[:1, :])

            # W1 as bf16 k-chunk tiles
            W1_bf = []
            for kc in range(NKT):
                wt = cst.tile([P, P], f32, tag="w1f")
                nc.sync.dma_start(wt[:], t_W1[kc * P:(kc + 1) * P, :])
                wb = cst.tile([P, P], bf16, tag=f"w1b{kc}")
                nc.vector.tensor_copy(wb[:], wt[:])
                W1_bf.append(wb)

            # ---- phase 1: h' shard (x pre-transposed on host, bf16) ----
            # two column groups; each group's x loads, matmuls, and its
            # AllGather (one per table half) pipeline so the collective
            # overlaps compute and the half-A gathers.
            h_in = dram.tile([S_SHARD, P], bf16)
            CH = S_SHARD // 2
            CHT = CH // P  # tiles per allgather chunk
            h_halves = [dram.tile([NCORES * CH, P], bf16, addr_space="Shared",
                                  tag=f"hfull{j}", name=f"h_half{j}")
                        for j in range(2)]
            with tc.tile_pool(name="xstr", bufs=1) as xsp:
                xbs = []
                for kc in range(NKT):
                    xb = xsp.tile([P, S_SHARD], bf16, tag=f"xb{kc}")
                    xbs.append(xb)
                xqs = [nc.sync, nc.scalar]
                for jj in range(2):
                    for kc in range(NKT):
                        xqs[kc % 2].dma_start(
                            xbs[kc][:, jj * CH:(jj + 1) * CH],
                            t_xT[kc * P:(kc + 1) * P, jj * CH:(jj + 1) * CH])
                for jj in range(2):
                    for tt in range(CHT):
                        t = jj * CHT + tt
                        hps = psB.tile([P, P], f32, tag="acc")
                        for kc in range(NKT):
                            nc.tensor.matmul(hps[:], lhsT=xbs[kc][:, t * P:(t + 1) * P],
                                             rhs=W1_bf[kc][:],
                                             start=(kc == 0), stop=(kc == NKT - 1))
                        hp = sm.tile([P, P], bf16, tag="hp")
                        nc.vector.tensor_scalar(out=hp[:], in0=hps[:],
                                                scalar1=dinvs_sb[:, t:t + 1], scalar2=None,
                                                op0=ALU.mult)
                        nc.sync.dma_start(h_in[t * P:(t + 1) * P, :], hp[:])
                    # allgather half jj as soon as its rows are written
                    nc.gpsimd.collective_compute(
                        "AllGather", ALU.bypass,
                        replica_groups=[list(range(NCORES))],
                        ins=[h_in[jj * CH:(jj + 1) * CH, :].opt()],
                        outs=[h_halves[jj].opt()],
                    )

            # phase-2 pools open after the x-streaming pool is released
            es = contextlib.ExitStack()
            gp = es.enter_context(tc.tile_pool(name="gat", bufs=1))
            ohp = es.enter_context(tc.tile_pool(name="ohp", bufs=1))

            # ---- phase 2: edge aggregation ----
            # gather ops per half: list of (chunk_base_slot, nchunks)
            def half_ops(L):
                ops = []
                base = 0
                while base < L:
                    n = min(OPCHUNKS * P, L - base)
                    ops.append((base, n))
                    base += n
                return ops

            opsA = half_ops(half_len[0])
            opsB = half_ops(half_len[1])
            gtiles = {0: {}, 1: {}}
            idx_sb = {0: idxA_sb, 1: idxB_sb}
            tabs = {0: h_halves[0][:], 1: h_halves[1][:]}

            gq_counter = [0]

            def issue_gather(hh, opi, base, nsl):
                g = gp.tile([P, OPCHUNKS * P], bf16, tag=f"g{hh}", bufs=GBUFS)
                nc.gpsimd.dma_gather(
                    g[:, :nsl].rearrange("p (c f) -> p c f", f=P),
                    tabs[hh],
                    idx_sb[hh][:, base // 16: (base + nsl) // 16],
                    nsl, nsl, P,
                    queue_num=gq_counter[0] % NQUEUES,
                )
                gq_counter[0] += 1
                gtiles[hh][opi] = g

            # issue gathers in consumption order (windows interleave the two
            # halves, so strict A-then-B issue order head-of-line blocks the
            # buffer rings)
            allops = []
            for hh, ops in ((0, opsA), (1, opsB)):
                for opi, (base, nsl) in enumerate(ops):
                    fw = int(np.searchsorted(seg_off[:, hh], base, side="right")) - 1
                    allops.append((fw, hh, opi, base, nsl))
            allops.sort()
            for fw, hh, opi, base, nsl in allops:
                issue_gather(hh, opi, base, nsl)

            # batched one-hot builds: one DVE op per OHG chunks
            oh_tiles = {}

            def onehot_group(g0):
                n = min(OHG, NPARTS - g0)
                oh = ohp.tile([P, OHG * P], bf16, tag="oh", bufs=10)
                nc.vector.tensor_tensor(
                    out=oh[:, :n * P].rearrange("p (c f) -> p c f", f=P),
                    in0=iota8_sb[:, :n * P].rearrange("p (c f) -> p c f", f=P),
                    in1=dcol_sb[:, g0:g0 + n].to_broadcast([P, n, P]),
                    op=ALU.is_equal)
                oh_tiles[g0] = oh

            winmax_sb = cst.tile([P, NWIN], f32)
            TRW = 4  # windows per transpose/reduce batch
            built_oh = set()

            def get_oh(ci):
                g0 = (ci // OHG) * OHG
                if g0 not in built_oh:
                    onehot_group(g0)
                    built_oh.add(g0)
                return oh_tiles[g0]

            pi = 0  # global part cursor into parts_tab / dstcol columns
            for w in range(NWIN):
                if w % TRW == 0:
                    tr = psA.tile([P, TRW * P], bf16, tag="tp")
                agg = psB.tile([P, P], f32, tag="acc")
                wparts = []
                while pi < NPARTS and parts_tab[pi][0] == w:
                    wparts.append((pi,) + tuple(parts_tab[pi][1:]))
                    pi += 1
                if need_bias:
                    # bias chunk: agg += sqrt(deg)[col] * b1[f]
                    nc.tensor.matmul(agg[:], lhsT=sdegT_bf[:1, w * P:(w + 1) * P],
                                     rhs=b1row_bf[:1, :], start=True, stop=False)
                for j, (pidx, hh, lc, a, b) in enumerate(wparts):
                    opi, off = divmod(lc * P, OPCHUNKS * P)
                    g = gtiles[hh][opi]
                    oh = get_oh(pidx)
                    ohc = (pidx % OHG) * P
                    nc.tensor.matmul(agg[:],
                                     lhsT=oh[:, ohc:ohc + P],
                                     rhs=g[:, off:off + P],
                                     start=(j == 0 and not need_bias),
                                     stop=(j == len(wparts) - 1))
                # hx = relu(dinv * (agg + sqrt(deg)*b1)) = relu(dinv*agg + b1)
                hx = sm.tile([P, P], bf16, tag="hx")
                nc.scalar.activation(hx[:], agg[:], AF.Relu,
                                     scale=dinvp_sb[:, w:w + 1])
                if need_mask:
                    nc.vector.tensor_scalar(out=hx[:], in0=hx[:],
                                            scalar1=mask_sb[:, w:w + 1], scalar2=None,
                                            op0=ALU.add)
                nc.tensor.transpose(tr[:, (w % TRW) * P:(w % TRW + 1) * P],
                                    hx[:], ident_bf[:])
                if w % TRW == TRW - 1:
                    nc.vector.reduce_max(
                        out=winmax_sb[:, w - TRW + 1:w + 1],
                        in_=tr[:].rearrange("p (c f) -> p c f", f=P), axis=AX.X)

            # ---- pooling: per-graph max over its windows ----
            pooled_sb = cst.tile([P, GPC], f32)
            for g in range(GPC):
                nc.vector.reduce_max(out=pooled_sb[:, g:g + 1],
                                     in_=winmax_sb[:, g * WPG:(g + 1) * WPG], axis=AX.X)

            # ---- news = relu(x_root @ W0 + b0) ----
            nps = psB.tile([GPC, P], f32, tag="acc")
            for kc in range(NKT):
                rt = sm.tile([P, GPC], f32, tag="rt")
                nc.sync.dma_start(rt[:], t_roots[kc * P:(kc + 1) * P, :])
                w0t = sm.tile([P, P], f32, tag="w0t")
                nc.sync.dma_start(w0t[:], t_W0[kc * P:(kc + 1) * P, :])
                nc.tensor.matmul(nps[:], lhsT=rt[:], rhs=w0t[:],
                                 start=(kc == 0), stop=(kc == NKT - 1))
            news = sm.tile([GPC, P], f32, tag="news")
            nc.vector.tensor_add(news[:], nps[:], b0_sb[:GPC, :])
            nc.scalar.activation(news[:], news[:], AF.Relu)
            ntr = psA.tile([P, GPC], f32, tag="tp")
            nc.tensor.transpose(ntr[:], news[:], ident_f[:GPC, :GPC])
            newsT = sm.tile([P, GPC], f32, tag="newsT")
            nc.vector.tensor_copy(newsT[:], ntr[:])

            # ---- z = relu([pooled|news] @ Wl1 + bl1) ----
            wl1a = sm.tile([P, P], f32, tag="wl1a")
            nc.sync.dma_start(wl1a[:], t_Wl1[0:P, :])
            wl1b = sm.tile([P, P], f32, tag="wl1b")
            nc.sync.dma_start(wl1b[:], t_Wl1[P:2 * P, :])
            zps = psB.tile([GPC, P], f32, tag="acc")
            nc.tensor.matmul(zps[:], lhsT=pooled_sb[:], rhs=wl1a[:], start=True, stop=False)
            nc.tensor.matmul(zps[:], lhsT=newsT[:], rhs=wl1b[:], start=False, stop=True)
            z2 = sm.tile([GPC, P], f32, tag="z2")
            nc.vector.tensor_add(z2[:], zps[:], bl1_sb[:GPC, :])
            nc.scalar.activation(z2[:], z2[:], AF.Relu)
            ztr = psA.tile([P, GPC], f32, tag="tp")
            nc.tensor.transpose(ztr[:], z2[:], ident_f[:GPC, :GPC])
            z2T = sm.tile([P, GPC], f32, tag="z2T")
            nc.vector.tensor_copy(z2T[:], ztr[:])

            # ---- logits + log_softmax ----
            wl2 = sm.tile([P, NCLS], f32, tag="wl2")
            nc.sync.dma_start(wl2[:], t_Wl2[:])
            lps = psB.tile([GPC, NCLS], f32, tag="acc")
            nc.tensor.matmul(lps[:], lhsT=z2T[:], rhs=wl2[:], start=True, stop=True)
            lg = sm.tile([GPC, NCLS], f32, tag="lg")
            nc.vector.tensor_add(lg[:], lps[:], bl2_sb[:GPC, :])
            mx = sm.tile([GPC, 1], f32, tag="mx")
            nc.vector.reduce_max(out=mx[:], in_=lg[:], axis=AX.X)
            tt = sm.tile([GPC, NCLS], f32, tag="tt")
            nc.vector.tensor_scalar(out=tt[:], in0=lg[:], scalar1=mx[:],
                                    scalar2=None, op0=ALU.subtract)
            ee = sm.tile([GPC, NCLS], f32, tag="ee")
            nc.scalar.activation(ee[:], tt[:], AF.Exp)
            ss = sm.tile([GPC, 1], f32, tag="ss")
            nc.vector.reduce_sum(out=ss[:], in_=ee[:], axis=AX.X)
            ls = sm.tile([GPC, 1], f32, tag="ls")
            nc.scalar.activation(ls[:], ss[:], AF.Ln)
            yy = sm.tile([GPC, NCLS], f32, tag="yy")
            nc.vector.tensor_scalar(out=yy[:], in0=tt[:], scalar1=ls[:],
                                    scalar2=None, op0=ALU.subtract)
            nc.sync.dma_start(t_y[:], yy[:])
            es.close()

    nc.compile()
    return nc


# ---------------------------------------------------------------------------
# entry point
# ---------------------------------------------------------------------------
def kernel(**inputs) -> np.ndarray:
    global LAST_EXEC_NS
    _install_ntff_hook()
    from concourse import bass_utils
    from concourse.bass_interp import get_hw_module

    x = np.asarray(inputs["x"], dtype=np.float32)
    ei = np.asarray(inputs["edge_index"])
    batch = np.asarray(inputs["batch"])
    G = int(np.asarray(inputs["num_graphs"]))
    W1 = np.asarray(inputs["W1"], dtype=np.float32)
    b1 = np.asarray(inputs["b1"], dtype=np.float32)
    W0 = np.asarray(inputs["W0"], dtype=np.float32)
    b0 = np.asarray(inputs["b0"], dtype=np.float32)
    Wl1 = np.asarray(inputs["Wl1"], dtype=np.float32)
    bl1 = np.asarray(inputs["bl1"], dtype=np.float32)
    Wl2 = np.asarray(inputs["Wl2"], dtype=np.float32)
    bl2 = np.asarray(inputs["bl2"], dtype=np.float32)
    NCLS = Wl2.shape[1]

    s = build_schedule(x, ei, batch, G)
    need_mask = bool((b1 > 0).any())
    need_bias = bool((b1 != 0).any())
    nc = build_program(s, NCLS, need_mask, need_bias)

    rep = lambda v, n: np.ascontiguousarray(np.tile(v[None, :], (n, 1)).astype(np.float32))
    iota_row = rep(np.arange(P, dtype=np.float32), P)
    iota8 = np.ascontiguousarray(np.tile(iota_row, (1, 8)))
    ident = np.eye(P, dtype=np.float32)

    in_maps = []
    for c in range(NCORES):
        in_maps.append({
            "xT_core": np.ascontiguousarray(s["x_core"][c].T).astype(ml_dtypes.bfloat16),
            "W1": W1, "W0": W0, "Wl1": Wl1, "Wl2": Wl2,
            "b1_rep": rep(b1, P), "b0_rep": rep(b0, P),
            "bl1_rep": rep(bl1, P), "bl2_rep": rep(bl2, P),
            "deg_shard": s["deg_shard"][c], "deg_pad": s["deg_pad"][c],
            "deg_padT": np.ascontiguousarray(s["deg_padT"][c].reshape(1, -1)),
            "maskneg": s["maskneg"][c], "rootsT": s["rootsT"][c],
            "idxA": s["idxA"][c], "idxB": s["idxB"][c],
            "dstcol": s["dstcol"][c],
            "iota_row": iota_row, "iota8": iota8, "identity": ident,
        })

    nc.m = get_hw_module(nc.m)
    res = bass_utils.run_bass_kernel_spmd(
        nc, in_maps, core_ids=list(range(NCORES)), trace=TRACE)
    LAST_EXEC_NS = res.exec_time_ns

    out = np.zeros((G, NCLS), np.float32)
    for c in range(NCORES):
        out[s["out_map"][c]] = res.results[c]["y"]
    return out

